# revision 1
# baseline (speedup 1.0000x reference)
"""AdSBHNet trapezoid-integral kernel for 8 TRN2 NeuronCores.

Math (all-real reformulation of the complex reference):
  poly(c,z) = sum_{i=1..5} c_i z^i ;  f = (1-z^4) e^{poly(a,z)} ; g = e^{poly(b,z)}/(1-z^4)
  z = zs*u on a uniform u-grid (Nu_L=2000 / Nu_V=1500), du == h everywhere.
  L: w  = A/(D+eps+i*eps) - 1 + eps(1+i),  A = zs^4 f(z), D = z^4 f(zs)
     integrand = sqrt(g)/sqrt(w);  L = (2/pi) * zs*h * sum(w_j * integrand_j)
  V: inner = 1 - Y/(X+eps+i*eps) + eps(1+i), Y = z^4 f(zs), X = zs^4 f(z)
     term = sqrt(f g)/sqrt(inner) - 1; integrand = term/(z^2+eps(1+i))
     V = 2pi*zs*h*sum(w_j integrand_j) - 2pi/zs
  Complex sqrt of w=re+i*im with r=|w|: sqrt(w) = p + i*q, p=sqrt((r+re)/2),
  q=sign(im)*sqrt((r-re)/2); 1/sqrt(w) = (p-i*q)/r.  For sqrt(g)/sqrt(w):
    sqrt(g)*p/r = sqrt(G*(r+re)), sqrt(g)*|q|/r = sqrt(G*(r-re)),
    G = g/(2 r^2) > 0.
  Numerical stability: r-|re| catastrophically cancels, so compute
    rlarge = r+|re| (well conditioned) and rsmall = im^2/rlarge
  (identity r^2-re^2 = im^2) and route sqrt(G*rlarge)/sqrt(G*rsmall) to the
  p/q slots by sign(re) with predicated copies — same branch structure as
  the reference's complex sqrt.

Polynomial evaluation over the [128,N] grid is a K=6 TensorE matmul:
  poly(c, zs_r*u_j) = sum_i (zs_r^i) * (c_i u_j^i);  lhsT = zs-powers [6,128],
  rhs = c-scaled u-powers [6,N]; row 0 (k=0) carries trapezoid ln-weights.

Sharding: pure data parallel, zs batch split 8 ways; a/b replicated.
"""

import math
import sys

import numpy as np

sys.path.insert(0, "/opt/trn_rl_repo")

import concourse.bass as bass
import concourse.bacc as bacc
import concourse.mybir as mybir
from concourse import bass_utils
from concourse.tile import TileContext

F32 = mybir.dt.float32
I32 = mybir.dt.int32
BF16 = mybir.dt.bfloat16
U16 = mybir.dt.uint16
OP = mybir.AluOpType
AF = mybir.ActivationFunctionType

EPS = 1e-6
EPS2 = EPS * EPS
NU_L = 2000
NU_V = 1500
B = 8192
NCORES = 8
BLOC = B // NCORES       # 1024 rows per core
NT = BLOC // 128         # 8 row-tiles per core
H_L = (1.0 - 2 * EPS) / (NU_L - 1)
H_V = (1.0 - 2 * EPS) / (NU_V - 1)
LN2 = math.log(2.0)
LNW2 = math.log(0.25)    # ln(w^2) at trapezoid endpoints (w=0.5)
MMC = 512                # matmul free-dim chunk


def _chunks(n):
    return [(c, min(c + MMC, n)) for c in range(0, n, MMC)]


def build_nc(reps=1):
    nc = bacc.Bacc("TRN2", target_bir_lowering=False, debug=False, num_devices=NCORES)
    a_d = nc.declare_dram_parameter("a", [5], F32, isOutput=False)
    b_d = nc.declare_dram_parameter("b", [5], F32, isOutput=False)
    zs_d = nc.declare_dram_parameter("zs", [BLOC], F32, isOutput=False)
    out_d = nc.declare_dram_parameter("out", [4, BLOC], F32, isOutput=True)

    with TileContext(nc) as tc:
        with (
            tc.tile_pool(name="cst", bufs=1) as cst,
            tc.tile_pool(name="wk", bufs=1) as wk,
            tc.tile_pool(name="ps", bufs=1, space="PSUM") as pspool,
        ):
            v = nc.vector
            sc = nc.scalar

            def W(tag, n=NU_L, dt=F32, nm=None):
                return wk.tile([128, n], dt, tag=tag, name=nm or f"t{tag}")

            # ---------------- setup: per-row quantities ----------------
            zcol = cst.tile([128, NT], F32)          # zs, col t = tile t
            nc.sync.dma_start(out=zcol[:], in_=zs_d[:].rearrange("(t p) -> p t", p=128))
            zrow = wk.tile([1, BLOC], F32, tag="a0", name="zrow")
            nc.sync.dma_start(out=zrow[:], in_=zs_d[:].rearrange("(o n) -> o n", o=1))

            aext = cst.tile([6, 1], F32)
            bext = cst.tile([6, 1], F32)
            v.memset(aext[:], 0.0)
            v.memset(bext[:], 0.0)
            nc.sync.dma_start(out=aext[1:6, 0:1], in_=a_d[:])
            nc.sync.dma_start(out=bext[1:6, 0:1], in_=b_d[:])
            abext = cst.tile([6, 1], F32)
            v.tensor_tensor(abext[:], aext[:], bext[:], OP.add)

            ones6 = cst.tile([1, 6], F32)
            v.memset(ones6[:], 1.0)

            # kcol6 = [0,1,2,3,4,5]; row 0 is the const-1 / weight row
            kcol_i = cst.tile([6, 1], I32)
            nc.gpsimd.iota(kcol_i[:], pattern=[[1, 1]], base=0, channel_multiplier=1)
            kcol6 = cst.tile([6, 1], F32)
            v.tensor_copy(kcol6[:], kcol_i[:])

            # ZPow6 [6, BLOC]: row k = zs^k (row 0 = 1) via exp(k ln zs)
            lnz = wk.tile([1, BLOC], F32, tag="a1", name="lnz")
            sc.activation(lnz[:], zrow[:], AF.Ln)
            ps6 = pspool.tile([6, BLOC], F32, tag="pa", name="ps6")
            for c0, c1 in _chunks(BLOC):
                nc.tensor.matmul(ps6[:, c0:c1], ones6[:], lnz[:, c0:c1], start=True, stop=True)
            klnz = wk.tile([6, BLOC], F32, tag="a2", name="klnz")
            v.tensor_scalar(klnz[:], ps6[:], kcol6[:], None, OP.mult)
            zpow = cst.tile([6, BLOC], F32)
            sc.activation(zpow[:], klnz[:], AF.Exp)

            # per-row [128, NT] tiles
            zs2c = cst.tile([128, NT], F32)
            v.tensor_tensor(zs2c[:], zcol[:], zcol[:], OP.mult)
            zs4c = cst.tile([128, NT], F32)
            v.tensor_tensor(zs4c[:], zs2c[:], zs2c[:], OP.mult)
            nzs4c = cst.tile([128, NT], F32)
            v.tensor_scalar(nzs4c[:], zs4c[:], -1.0, None, OP.mult)
            lnzs4 = cst.tile([128, NT], F32)
            sc.activation(lnzs4[:], zs4c[:], AF.Ln)

            # pa(zs) for all rows via 8 tiny matmuls -> [128, NT]
            ps_pz = pspool.tile([128, NT], F32, tag="pb", name="ps_pz")
            for t in range(NT):
                nc.tensor.matmul(
                    ps_pz[:, t : t + 1], zpow[:, t * 128 : (t + 1) * 128], aext[:],
                    start=True, stop=True,
                )
            e_paz = cst.tile([128, NT], F32)
            sc.activation(e_paz[:], ps_pz[:], AF.Exp)
            omzs4 = cst.tile([128, NT], F32)
            v.tensor_scalar(omzs4[:], zs4c[:], -1.0, 1.0, OP.mult, OP.add)
            fzs = cst.tile([128, NT], F32)
            v.tensor_tensor(fzs[:], e_paz[:], omzs4[:], OP.mult)
            c1c = cst.tile([128, NT], F32)
            v.tensor_tensor(c1c[:], zs4c[:], fzs[:], OP.mult)

            # scales
            sL = cst.tile([128, NT], F32)
            v.tensor_scalar(sL[:], zcol[:], 2.0 * H_L / math.pi, None, OP.mult)
            sLn = cst.tile([128, NT], F32)
            v.tensor_scalar(sLn[:], zcol[:], -2.0 * H_L / math.pi, None, OP.mult)
            sV = cst.tile([128, NT], F32)
            v.tensor_scalar(sV[:], zcol[:], 2.0 * math.pi * H_V, None, OP.mult)
            sVn = cst.tile([128, NT], F32)
            v.tensor_scalar(sVn[:], zcol[:], -2.0 * math.pi * H_V, None, OP.mult)
            invz = cst.tile([128, NT], F32)
            invz_s = cst.tile([128, NT], F32)
            v.reciprocal_approx_accurate(invz[:], zcol[:], invz_s[:])

            # ---------------- setup: u-grids ----------------
            io_c = W("a0", dt=I32, nm="io_c")
            nc.gpsimd.iota(io_c[:], pattern=[[1, NU_L]], base=0, channel_multiplier=0)
            iof = W("a1", nm="iof")
            v.tensor_copy(iof[:], io_c[:])
            io6_c = wk.tile([6, NU_L], I32, tag="a2", name="io6_c")
            nc.gpsimd.iota(io6_c[:], pattern=[[1, NU_L]], base=0, channel_multiplier=0)
            iof6 = wk.tile([6, NU_L], F32, tag="a3", name="iof6")
            v.tensor_copy(iof6[:], io6_c[:])

            grids = {}
            for gname, N, H in (("L", NU_L, H_L), ("V", NU_V, H_V)):
                u1 = W("a4", N, nm=f"u1{gname}")
                v.tensor_scalar(u1[:], iof[:, 0:N], H, EPS, OP.mult, OP.add)
                if gname == "V":
                    u2 = cst.tile([128, N], F32, name="u2V")
                else:
                    u2 = W("a5", N, nm="u2L")
                v.tensor_tensor(u2[:], u1[:], u1[:], OP.mult)
                u4 = cst.tile([128, N], F32, name=f"u4{gname}")
                v.tensor_tensor(u4[:], u2[:], u2[:], OP.mult)

                # Upow6 [6,N] = u^k rows (row0 = 1) via exp(k ln u)
                u16 = wk.tile([6, N], F32, tag="a6", name=f"u16{gname}")
                v.tensor_scalar(u16[:], iof6[:, 0:N], H, EPS, OP.mult, OP.add)
                lnu = wk.tile([6, N], F32, tag="a7", name=f"lnu{gname}")
                sc.activation(lnu[:], u16[:], AF.Ln)
                klnu = wk.tile([6, N], F32, tag="a9", name=f"klnu{gname}")
                v.tensor_scalar(klnu[:], lnu[:], kcol6[:], None, OP.mult)
                upow = wk.tile([6, N], F32, tag="aB", name=f"upow{gname}")
                sc.activation(upow[:], klnu[:], AF.Exp)

                ra_t = cst.tile([6, N], F32, name=f"ra{gname}")
                ra = ra_t[:]
                v.tensor_scalar(ra, upow[:], aext[:], None, OP.mult)
                if gname == "L":
                    rb_t = cst.tile([6, N], F32, name="rbL")
                    rb = rb_t[:]
                    v.tensor_scalar(rb, upow[:], bext[:], None, OP.mult)
                    # trapezoid endpoint ln-weights in row 0 of rb:
                    # iota = p + j (resp. p + N-1-j) is 0 only at the target elem
                    nc.gpsimd.affine_select(
                        out=rb, in_=rb, pattern=[[1, N]],
                        compare_op=OP.is_gt, fill=LNW2, base=0,
                        channel_multiplier=1,
                    )
                    nc.gpsimd.affine_select(
                        out=rb, in_=rb, pattern=[[-1, N]],
                        compare_op=OP.is_gt, fill=LNW2, base=N - 1,
                        channel_multiplier=1,
                    )
                    grids["L"] = (u4, ra, rb)
                else:
                    rab_t = cst.tile([6, N], F32, name="rabV")
                    rab = rab_t[:]
                    v.tensor_scalar(rab, upow[:], abext[:], None, OP.mult)
                    grids["V"] = (u2, u4, ra, rab)

            # accumulators & scratch
            accLre = cst.tile([128, NT], F32)
            accLim = cst.tile([128, NT], F32)
            accVre = cst.tile([128, NT], F32)
            accVim = cst.tile([128, NT], F32)
            dum = cst.tile([128, NU_L], F32)   # ACT accum scratch, never read
            nln2 = cst.tile([128, 1], F32)     # -ln2 bias column
            v.memset(nln2[:], -LN2)
            nhln2 = cst.tile([128, 1], F32)    # -ln2/2 bias column
            v.memset(nhln2[:], -0.5 * LN2)
            c_one = cst.tile([128, 1], F32)
            v.memset(c_one[:], 1.0)
            c_negk = cst.tile([128, 1], F32)   # -(1-eps)
            v.memset(c_negk[:], -(1.0 - EPS))
            c_eps = cst.tile([128, 1], F32)
            v.memset(c_eps[:], EPS)
            c_onep = cst.tile([128, 1], F32)   # 1+eps
            v.memset(c_onep[:], 1.0 + EPS)

            # ---------------- main loop ----------------
            U4L, RA_L, RB_L = grids["L"]
            U2V, U4V, RA_V, RAB_V = grids["V"]

            for rep in range(reps):
              for t in range(NT):
                lhs = zpow[:, t * 128 : (t + 1) * 128]
                nzs4_t = nzs4c[:, t : t + 1]
                c1_t = c1c[:, t : t + 1]
                ln4_t = lnzs4[:, t : t + 1]

                # ======== L integral (N=2000) ========
                N = NU_L
                pa_ps = pspool.tile([128, N], F32, tag="pa", name="paL")
                pb_ps = pspool.tile([128, N], F32, tag="pb", name="pbL")
                for c0, c1 in _chunks(N):
                    nc.tensor.matmul(pa_ps[:, c0:c1], lhs, RA_L[:, c0:c1], start=True, stop=True)
                for c0, c1 in _chunks(N):
                    nc.tensor.matmul(pb_ps[:, c0:c1], lhs, RB_L[:, c0:c1], start=True, stop=True)

                e_a2 = W("a0")
                sc.activation(e_a2[:], pa_ps[:], AF.Exp, bias=ln4_t, scale=1.0)
                omz4 = W("a2")
                v.tensor_scalar(omz4[:], U4L[:], nzs4_t, 1.0, OP.mult, OP.add)
                Dp = W("a3")
                v.tensor_scalar(Dp[:], U4L[:], c1_t, EPS, OP.mult, OP.add)
                X = W("a4")
                nc.gpsimd.tensor_tensor(X[:], omz4[:], e_a2[:], OP.mult)
                sqDp = W("a5")
                nc.gpsimd.tensor_tensor(sqDp[:], Dp[:], Dp[:], OP.mult)
                n2 = W("a6")
                v.tensor_scalar(n2[:], sqDp[:], EPS2, None, OP.add)
                rn2 = W("a5")
                v.reciprocal_approx_fast(rn2[:], n2[:])
                t_ = W("a6")
                v.tensor_tensor(t_[:], X[:], rn2[:], OP.mult)
                tDp = W("a4")
                v.tensor_tensor(tDp[:], t_[:], Dp[:], OP.mult)
                # re = tDp-(1-eps), im = eps*(1-t_): folded into ACT affine below
                sgn = W("a8", dt=BF16)
                sc.activation(sgn[:], t_[:], AF.Sign, bias=c_one[:, 0:1], scale=-1.0)
                sqre = W("a5")
                sc.activation(sqre[:], tDp[:], AF.Square, bias=c_negk[:, 0:1], scale=1.0)
                sqim = W("a9")
                sc.activation(sqim[:], t_[:], AF.Square, bias=c_eps[:, 0:1], scale=-EPS)
                r2s = W("a6")
                nc.gpsimd.tensor_tensor(r2s[:], sqre[:], sqim[:], OP.add)
                lnom = W("a7")
                sc.activation(lnom[:], omz4[:], AF.Ln)
                lnr2s = W("a5")
                sc.activation(lnr2s[:], r2s[:], AF.Ln)
                r_ = W("aA", dt=BF16)
                sc.activation(r_[:], lnr2s[:], AF.Exp, scale=0.5)
                absre = W("a2", dt=BF16)
                sc.activation(absre[:], tDp[:], AF.Abs, bias=c_negk[:, 0:1], scale=1.0)
                rlg = W("a3", dt=BF16)
                v.tensor_tensor(rlg[:], absre[:], r_[:], OP.add)
                lnrlg = W("aB")
                sc.activation(lnrlg[:], rlg[:], AF.Ln)
                base = W("a0")
                v.tensor_tensor(base[:], pb_ps[:], lnom[:], OP.subtract)
                base2 = W("a2")
                v.tensor_tensor(base2[:], base[:], lnr2s[:], OP.subtract)
                lnim2 = W("a6")
                sc.activation(lnim2[:], sqim[:], AF.Ln)
                lnglg = W("a5")
                v.tensor_tensor(lnglg[:], base2[:], lnrlg[:], OP.add)
                SS = W("a1", dt=BF16)            # -> becomes igq after swap
                sc.activation(SS[:], lnglg[:], AF.Exp, bias=nhln2[:, 0:1], scale=0.5)
                prt = W("a3")
                nc.gpsimd.tensor_tensor(prt[:], base2[:], lnrlg[:], OP.subtract)
                lngsm = W("a0")
                v.tensor_tensor(lngsm[:], prt[:], lnim2[:], OP.add)
                TTs = W("a7", dt=BF16)           # -> becomes igre after swap
                sc.activation(TTs[:], lngsm[:], AF.Exp, bias=nhln2[:, 0:1], scale=0.5)
                TTs2 = W("a9", dt=BF16)
                v.tensor_copy(TTs2[:], TTs[:])
                m = W("aA", dt=BF16)
                v.tensor_scalar(m[:], tDp[:], 1.0 - EPS, None, OP.is_ge)
                # igre = m ? SS : TTs ; igq = m ? TTs : SS
                v.copy_predicated(TTs[:], m[:].bitcast(U16), SS[:])
                v.copy_predicated(SS[:], m[:].bitcast(U16), TTs2[:])
                sc.activation(dum[:, 0:N], TTs[:], AF.Copy,
                              accum_out=accLre[:, t : t + 1])
                igqs = W("a2", dt=BF16)
                v.scalar_tensor_tensor(
                    igqs[:], SS[:], 1.0, sgn[:], OP.mult, OP.mult,
                    accum_out=accLim[:, t : t + 1],
                )

                # ======== V integral (N=1500) ========
                N = NU_V
                pa_ps = pspool.tile([128, N], F32, tag="pa", name="paV")
                pab_ps = pspool.tile([128, N], F32, tag="pb", name="pabV")
                for c0, c1 in _chunks(N):
                    nc.tensor.matmul(pa_ps[:, c0:c1], lhs, RA_V[:, c0:c1], start=True, stop=True)
                for c0, c1 in _chunks(N):
                    nc.tensor.matmul(pab_ps[:, c0:c1], lhs, RAB_V[:, c0:c1], start=True, stop=True)

                e_a2 = W("b0", N)
                sc.activation(e_a2[:], pa_ps[:], AF.Exp, bias=ln4_t, scale=1.0)
                omz4 = W("b2", N)
                v.tensor_scalar(omz4[:], U4V[:], nzs4_t, 1.0, OP.mult, OP.add)
                Y = W("b3", N)
                v.tensor_scalar(Y[:], U4V[:], c1_t, None, OP.mult)
                X = W("b4", N)
                nc.gpsimd.tensor_tensor(X[:], omz4[:], e_a2[:], OP.mult)
                Xp = W("b0", N)
                v.tensor_scalar(Xp[:], X[:], EPS, None, OP.add)
                sqXp = W("b2", N)
                nc.gpsimd.tensor_tensor(sqXp[:], Xp[:], Xp[:], OP.mult)
                n2v = W("b4", N)
                v.tensor_scalar(n2v[:], sqXp[:], EPS2, None, OP.add)
                rn2v = W("b2", N)
                v.reciprocal_approx_fast(rn2v[:], n2v[:])
                t2 = W("b4", N)
                v.tensor_tensor(t2[:], Y[:], rn2v[:], OP.mult)
                t2Xp = W("b3", N)
                v.tensor_tensor(t2Xp[:], t2[:], Xp[:], OP.mult)
                # re2 = 1+eps-t2Xp, im2 = eps*t2+eps: folded into ACT affine
                sqre2 = W("b0", N)
                sc.activation(sqre2[:], t2Xp[:], AF.Square, bias=c_onep[:, 0:1], scale=-1.0)
                sqim2 = W("b2", N)
                sc.activation(sqim2[:], t2[:], AF.Square, bias=c_eps[:, 0:1], scale=EPS)
                r2s2 = W("b4", N)
                nc.gpsimd.tensor_tensor(r2s2[:], sqre2[:], sqim2[:], OP.add)
                lnr2s2 = W("b0", N)
                sc.activation(lnr2s2[:], r2s2[:], AF.Ln)
                r2v = W("b5", N, dt=BF16)
                sc.activation(r2v[:], lnr2s2[:], AF.Exp, scale=0.5)
                absre2 = W("b6", N, dt=BF16)
                sc.activation(absre2[:], t2Xp[:], AF.Abs, bias=c_onep[:, 0:1], scale=-1.0)
                rlg2 = W("b1", N, dt=BF16)
                v.tensor_tensor(rlg2[:], absre2[:], r2v[:], OP.add)
                lnrlg2 = W("b5", N)
                sc.activation(lnrlg2[:], rlg2[:], AF.Ln)
                base2v = W("b2", N)
                v.tensor_tensor(base2v[:], pab_ps[:], lnr2s2[:], OP.subtract)
                lnim2v = W("b6", N)
                sc.activation(lnim2v[:], sqim2[:], AF.Ln)
                lnglg2 = W("b4", N)
                v.tensor_tensor(lnglg2[:], base2v[:], lnrlg2[:], OP.add)
                SSv = W("b0", N)                 # -> becomes M2 after swap
                sc.activation(SSv[:], lnglg2[:], AF.Exp, bias=nhln2[:, 0:1], scale=0.5)
                prt_v = W("b1", N)
                nc.gpsimd.tensor_tensor(prt_v[:], base2v[:], lnrlg2[:], OP.subtract)
                lngsm2 = W("b4", N)
                v.tensor_tensor(lngsm2[:], prt_v[:], lnim2v[:], OP.add)
                TTv = W("b2", N)                 # -> becomes P2 after swap
                sc.activation(TTv[:], lngsm2[:], AF.Exp, bias=nhln2[:, 0:1], scale=0.5)
                TTv2 = W("b5", N)
                v.tensor_copy(TTv2[:], TTv[:])
                m2 = W("b1", N, dt=BF16)
                v.tensor_scalar(m2[:], t2Xp[:], 1.0 + EPS, None, OP.is_le)
                # P2 = m2 ? SSv : TTv ; M2 = m2 ? TTv : SSv
                v.copy_predicated(TTv[:], m2[:].bitcast(U16), SSv[:])
                v.copy_predicated(SSv[:], m2[:].bitcast(U16), TTv2[:])
                P2 = TTv
                M2 = SSv

                zdb = W("b3", N, dt=BF16)
                v.tensor_scalar(zdb[:], U2V[:], zs2c[:, t : t + 1], EPS, OP.mult, OP.add)
                sqzd = W("b4", N)
                sc.activation(sqzd[:], zdb[:], AF.Square)
                ndn = W("b5", N)
                v.tensor_scalar(ndn[:], sqzd[:], EPS2, None, OP.add)
                lnndn = W("b4", N)
                sc.activation(lnndn[:], ndn[:], AF.Ln)
                rnd = W("b5", N)
                sc.activation(rnd[:], lnndn[:], AF.Exp, scale=-1.0)
                # endpoint trapezoid weights ride on rnd (shared by re & im)
                v.tensor_scalar(rnd[:, 0:1], rnd[:, 0:1], 0.5, None, OP.mult)
                v.tensor_scalar(rnd[:, N - 1 : N], rnd[:, N - 1 : N], 0.5, None, OP.mult)

                P2m = W("b6", N, dt=BF16)
                v.tensor_scalar(P2m[:], P2[:], -1.0, None, OP.add)
                M2b = W("b4", N, dt=BF16)
                sc.activation(M2b[:], M2[:], AF.Copy)
                A12 = W("b1", N, dt=BF16)
                v.tensor_tensor(A12[:], P2m[:], zdb[:], OP.mult)
                A4 = W("b2", N, dt=BF16)
                v.scalar_tensor_tensor(A4[:], M2b[:], -EPS, A12[:], OP.mult, OP.add)
                igre = W("b1", N)
                v.scalar_tensor_tensor(
                    igre[:], A4[:], 1.0, rnd[:], OP.mult, OP.mult,
                    accum_out=accVre[:, t : t + 1],
                )
                B1 = W("b0", N, dt=BF16)
                v.tensor_tensor(B1[:], M2b[:], zdb[:], OP.mult)
                B3 = W("b3", N, dt=BF16)
                v.scalar_tensor_tensor(B3[:], P2m[:], EPS, B1[:], OP.mult, OP.add)
                igim = W("b2", N)
                v.scalar_tensor_tensor(
                    igim[:], B3[:], 1.0, rnd[:], OP.mult, OP.mult,
                    accum_out=accVim[:, t : t + 1],
                )

            # ---------------- finals ----------------
            Lre_f = cst.tile([128, NT], F32)
            v.tensor_tensor(Lre_f[:], accLre[:], sL[:], OP.mult)
            Lim_f = cst.tile([128, NT], F32)
            v.tensor_tensor(Lim_f[:], accLim[:], sLn[:], OP.mult)
            Vraw = cst.tile([128, NT], F32)
            v.tensor_tensor(Vraw[:], accVre[:], sV[:], OP.mult)
            Vre_f = cst.tile([128, NT], F32)
            v.scalar_tensor_tensor(Vre_f[:], invz[:], -2.0 * math.pi, Vraw[:], OP.mult, OP.add)
            Vim_f = cst.tile([128, NT], F32)
            v.tensor_tensor(Vim_f[:], accVim[:], sVn[:], OP.mult)

            for row, tile in ((0, Lre_f), (1, Lim_f), (2, Vre_f), (3, Vim_f)):
                nc.sync.dma_start(
                    out=out_d[row, :].rearrange("(t p) -> p t", p=128), in_=tile[:]
                )
    return nc


_NC_CACHE = {}


def _restrict_act_tables(nc):
    """Monkeypatch table-set selection to the one set that serves every
    activation this kernel uses (exp/ln/square/sign/abs/copy/identity) so
    the steady state has zero ACT_TABLE_LOADs."""
    import types
    from concourse.hw_specs import get_activation_tables

    def _patched(self):
        # keep full list length so act_func_set_id indices stay aligned with
        # act_info.json; empty the other sets so only this one can be chosen
        tables = [(k, (v if k == "natural_log_exp_and_others" else set()))
                  for k, v in get_activation_tables(self.m.arch).items()]
        bacc._bass_rust.insert_act_table_loads(self, tables)

    nc.insert_act_table_loads = types.MethodType(_patched, nc)


def kernel(a, b, zs):
    a = np.asarray(a, dtype=np.float32)
    b = np.asarray(b, dtype=np.float32)
    zs = np.asarray(zs, dtype=np.float32)
    if "nc" not in _NC_CACHE:
        nc0 = build_nc()
        nc0.finalize()
        _NC_CACHE["nc"] = nc0
    nc = _NC_CACHE["nc"]
    in_maps = [
        {"a": a, "b": b, "zs": zs[i * BLOC : (i + 1) * BLOC].copy()}
        for i in range(NCORES)
    ]
    res = bass_utils.run_bass_kernel_spmd(nc, in_maps, core_ids=list(range(NCORES)))
    out = np.concatenate([res.results[i]["out"] for i in range(NCORES)], axis=1)
    return out.astype(np.float32)


if __name__ == "__main__":
    rng = np.random.default_rng(0)
    out = kernel(
        rng.standard_normal(5).astype(np.float32),
        rng.standard_normal(5).astype(np.float32),
        (0.02 + 0.975 * rng.random(8192)).astype(np.float32),
    )
    print(out.shape, out.dtype, out[:, :3])



# revision 3
# speedup vs baseline: 5.7002x; 5.7002x over previous
"""AdSBHNet trapezoid-integral kernel for 8 TRN2 NeuronCores.

Math (all-real reformulation of the complex reference):
  poly(c,z) = sum_{i=1..5} c_i z^i ;  f = (1-z^4) e^{poly(a,z)} ; g = e^{poly(b,z)}/(1-z^4)
  z = zs*u.
  L: w  = A/(D+eps+i*eps) - 1 + eps(1+i),  A = zs^4 f(z), D = z^4 f(zs)
     integrand = sqrt(g)/sqrt(w);  L = (2/pi) * zs * sum_j(w_j * integrand_j)
  V: inner = 1 - Y/(X+eps+i*eps) + eps(1+i), Y = z^4 f(zs), X = zs^4 f(z)
     term = sqrt(f g)/sqrt(inner) - 1; integrand = term/(z^2+eps(1+i))
     V = 2pi*zs*sum_j(w_j integrand_j) - 2pi/zs
  Complex sqrt of w=re+i*im with r=|w|: sqrt(w) = p + i*q, p=sqrt((r+re)/2),
  q=sign(im)*sqrt((r-re)/2); 1/sqrt(w) = (p-i*q)/r.  For sqrt(g)/sqrt(w):
    sqrt(g)*p/r = sqrt(G*(r+re)), sqrt(g)*|q|/r = sqrt(G*(r-re)),
    G = g/(2 r^2) > 0.
  Numerical stability: r-|re| catastrophically cancels, so compute
    rlarge = r+|re| (well conditioned) and rsmall = im^2/rlarge
  (identity r^2-re^2 = im^2) and route sqrt(G*rlarge)/sqrt(G*rsmall) to the
  p/q slots by sign(re) with predicated copies.

Quadrature: the reference's uniform trapezoid sums (2000/1500 points) are
replaced by an equivalent mixed rule validated to ~3e-6 relative against the
exact trapezoid sums:
  - the smooth middle of the u-range is integrated with 64-pt Gauss-Legendre
    (the trapezoid sum there equals the integral to ~1e-6 by Euler-Maclaurin),
  - the last 64 trapezoid points (eps-regularized endpoint singularity at
    u->1) are kept exactly,
  - for V additionally the first 64 trapezoid points (the z^2+eps peak near
    u->0) are kept exactly.
This gives 128 columns for L and 192 for V instead of 2000/1500.
Node positions/weights are baked into the NEFF as inline const tensors;
weights enter L via the 2*ln(w) row-0 trick of the pb matmul and V via a
direct multiply on the 1/(z^2+eps) factor.

Polynomial evaluation over the [128,N] grid is a K=6 TensorE matmul:
  poly(c, zs_r*u_j) = sum_i (zs_r^i) * (c_i u_j^i);  lhsT = zs-powers [6,128],
  rhs = c-scaled u-powers [6,N]; row 0 (k=0) carries quadrature ln-weights.

Sharding: pure data parallel, zs batch split 8 ways; a/b replicated.
"""

import math
import sys

import numpy as np

sys.path.insert(0, "/opt/trn_rl_repo")

import concourse.bass as bass
import concourse.bacc as bacc
import concourse.mybir as mybir
from concourse import bass_utils
from concourse.tile import TileContext

F32 = mybir.dt.float32
I32 = mybir.dt.int32
BF16 = mybir.dt.bfloat16
U16 = mybir.dt.uint16
OP = mybir.AluOpType
AF = mybir.ActivationFunctionType
AX = mybir.AxisListType

EPS = 1e-6
EPS2 = EPS * EPS
NU_L = 2000
NU_V = 1500
B = 8192
NCORES = 8
BLOC = B // NCORES       # 1024 rows per core
NT = BLOC // 128         # 8 row-tiles per core
H_L = (1.0 - 2 * EPS) / (NU_L - 1)
H_V = (1.0 - 2 * EPS) / (NU_V - 1)
LN2 = math.log(2.0)

N_GL = 64
TAIL_L = 63
HEAD_V = 63
TAIL_V = 63
NL = N_GL + TAIL_L + 1   # 128
NV = (HEAD_V + 1) + N_GL + (TAIL_V + 1)  # 192


def _mixed_nodes(Nu, h, n_head, n_gl, n_tail):
    """Nodes/weights replicating the Nu-pt trapezoid sum h*(f0/2+...+fN/2)
    with Gauss-Legendre on the smooth middle (float64)."""
    u = EPS + h * np.arange(Nu)
    nodes, wts = [], []
    if n_head:
        nodes.append(u[: n_head + 1])
        w = np.full(n_head + 1, h)
        w[0] = w[-1] = h / 2
        wts.append(w)
    lo = u[n_head]
    hi = u[Nu - 1 - n_tail]
    x, w = np.polynomial.legendre.leggauss(n_gl)
    nodes.append(0.5 * (hi + lo) + 0.5 * (hi - lo) * x)
    wts.append(0.5 * (hi - lo) * w)
    nodes.append(u[Nu - 1 - n_tail:])
    wt = np.full(n_tail + 1, h)
    wt[0] = wt[-1] = h / 2
    wts.append(wt)
    return np.concatenate(nodes), np.concatenate(wts)


_UL, _WL = _mixed_nodes(NU_L, H_L, 0, N_GL, TAIL_L)
_UV, _WV = _mixed_nodes(NU_V, H_V, HEAD_V, N_GL, TAIL_V)
assert len(_UL) == NL and len(_UV) == NV

_K6 = np.arange(6.0)
_UPOW_L = (_UL[None, :] ** _K6[:, None]).astype(np.float32)      # [6, NL]
_UPOW_V = (_UV[None, :] ** _K6[:, None]).astype(np.float32)      # [6, NV]
_LNW2_L = (2.0 * np.log(_WL))[None, :].astype(np.float32)        # [1, NL]
# broadcast row: [u4L | u2V | u4V | wV] -> [1, NL + 3*NV]
_BCROW = np.concatenate(
    [_UL**4, _UV**2, _UV**4, _WV]
)[None, :].astype(np.float32)
NBC = NL + 3 * NV


def build_nc(reps=1):
    nc = bacc.Bacc("TRN2", target_bir_lowering=False, debug=False, num_devices=NCORES)
    a_d = nc.declare_dram_parameter("a", [5], F32, isOutput=False)
    b_d = nc.declare_dram_parameter("b", [5], F32, isOutput=False)
    zs_d = nc.declare_dram_parameter("zs", [BLOC], F32, isOutput=False)
    out_d = nc.declare_dram_parameter("out", [4, BLOC], F32, isOutput=True)

    upowL_d = nc.inline_tensor(_UPOW_L, name="upowL")
    upowV_d = nc.inline_tensor(_UPOW_V, name="upowV")
    lnw2L_d = nc.inline_tensor(_LNW2_L, name="lnw2L")
    bcrow_d = nc.inline_tensor(_BCROW, name="bcrow")

    with TileContext(nc) as tc:
        with (
            tc.tile_pool(name="cst", bufs=1) as cst,
            tc.tile_pool(name="wk", bufs=2) as wk,
            tc.tile_pool(name="ps", bufs=2, space="PSUM") as pspool,
        ):
            v = nc.vector
            sc = nc.scalar
            gp = nc.gpsimd

            def W(tag, n=NL, dt=F32, nm=None):
                return wk.tile([128, n], dt, tag=tag, name=nm or f"t{tag}")

            # ---------------- setup: per-row quantities ----------------
            zcol = cst.tile([128, NT], F32)          # zs, col t = tile t
            nc.sync.dma_start(out=zcol[:], in_=zs_d[:].rearrange("(t p) -> p t", p=128))
            zrow = wk.tile([1, BLOC], F32, tag="a0", name="zrow")
            nc.sync.dma_start(out=zrow[:], in_=zs_d[:].rearrange("(o n) -> o n", o=1))

            aext = cst.tile([6, 1], F32)
            bext = cst.tile([6, 1], F32)
            v.memset(aext[:], 0.0)
            v.memset(bext[:], 0.0)
            nc.sync.dma_start(out=aext[1:6, 0:1], in_=a_d[:])
            nc.sync.dma_start(out=bext[1:6, 0:1], in_=b_d[:])
            abext = cst.tile([6, 1], F32)
            v.tensor_tensor(abext[:], aext[:], bext[:], OP.add)

            ones6 = cst.tile([1, 6], F32)
            v.memset(ones6[:], 1.0)
            ones128 = cst.tile([1, 128], F32)
            v.memset(ones128[:], 1.0)

            # kcol6 = [0,1,2,3,4,5]
            kcol_i = cst.tile([6, 1], I32)
            nc.gpsimd.iota(kcol_i[:], pattern=[[1, 1]], base=0, channel_multiplier=1)
            kcol6 = cst.tile([6, 1], F32)
            v.tensor_copy(kcol6[:], kcol_i[:])

            # u-power tables and weight row (inline consts)
            upowL = cst.tile([6, NL], F32)
            nc.sync.dma_start(out=upowL[:], in_=upowL_d[:, :])
            upowV = cst.tile([6, NV], F32)
            nc.sync.dma_start(out=upowV[:], in_=upowV_d[:, :])
            bcrow = wk.tile([1, NBC], F32, tag="a1", name="bcrow")
            nc.sync.dma_start(out=bcrow[:], in_=bcrow_d[:, :])

            # broadcast [1, NBC] across 128 partitions via K=1 matmul
            bc_sb = cst.tile([128, NBC], F32)
            for c0 in range(0, NBC, 512):
                c1 = min(c0 + 512, NBC)
                ps_bc = pspool.tile([128, c1 - c0], F32, tag="pa", name=f"psbc{c0}")
                nc.tensor.matmul(ps_bc[:], ones128[:], bcrow[:, c0:c1],
                                 start=True, stop=True)
                v.tensor_copy(bc_sb[:, c0:c1], ps_bc[:])
            u4Lb = bc_sb[:, 0:NL]
            u2Vb = bc_sb[:, NL:NL + NV]
            u4Vb = bc_sb[:, NL + NV:NL + 2 * NV]
            wVb = bc_sb[:, NL + 2 * NV:NBC]

            # ZPow6 [6, BLOC]: row k = zs^k (row 0 = 1) via exp(k ln zs)
            lnz = wk.tile([1, BLOC], F32, tag="a2", name="lnz")
            sc.activation(lnz[:], zrow[:], AF.Ln)
            ps6 = pspool.tile([6, BLOC], F32, tag="pb", name="ps6")
            for c0 in range(0, BLOC, 512):
                nc.tensor.matmul(ps6[:, c0:c0 + 512], ones6[:], lnz[:, c0:c0 + 512],
                                 start=True, stop=True)
            klnz = wk.tile([6, BLOC], F32, tag="a3", name="klnz")
            v.tensor_scalar(klnz[:], ps6[:], kcol6[:], None, OP.mult)
            zpow = cst.tile([6, BLOC], F32)
            sc.activation(zpow[:], klnz[:], AF.Exp)

            # per-row [128, NT] tiles
            zs2c = cst.tile([128, NT], F32)
            v.tensor_tensor(zs2c[:], zcol[:], zcol[:], OP.mult)
            zs4c = cst.tile([128, NT], F32)
            v.tensor_tensor(zs4c[:], zs2c[:], zs2c[:], OP.mult)
            nzs4c = cst.tile([128, NT], F32)
            v.tensor_scalar(nzs4c[:], zs4c[:], -1.0, None, OP.mult)
            lnzs4 = cst.tile([128, NT], F32)
            sc.activation(lnzs4[:], zs4c[:], AF.Ln)

            # pa(zs) for all rows via 8 tiny matmuls -> [128, NT]
            ps_pz = pspool.tile([128, NT], F32, tag="pc", name="ps_pz")
            for t in range(NT):
                nc.tensor.matmul(
                    ps_pz[:, t: t + 1], zpow[:, t * 128: (t + 1) * 128], aext[:],
                    start=True, stop=True,
                )
            e_paz = cst.tile([128, NT], F32)
            sc.activation(e_paz[:], ps_pz[:], AF.Exp)
            omzs4 = cst.tile([128, NT], F32)
            v.tensor_scalar(omzs4[:], zs4c[:], -1.0, 1.0, OP.mult, OP.add)
            fzs = cst.tile([128, NT], F32)
            v.tensor_tensor(fzs[:], e_paz[:], omzs4[:], OP.mult)
            c1c = cst.tile([128, NT], F32)
            v.tensor_tensor(c1c[:], zs4c[:], fzs[:], OP.mult)

            # scales (quadrature weights are baked per-column; only zs here)
            sL = cst.tile([128, NT], F32)
            v.tensor_scalar(sL[:], zcol[:], 2.0 / math.pi, None, OP.mult)
            sLn = cst.tile([128, NT], F32)
            v.tensor_scalar(sLn[:], zcol[:], -2.0 / math.pi, None, OP.mult)
            sV = cst.tile([128, NT], F32)
            v.tensor_scalar(sV[:], zcol[:], 2.0 * math.pi, None, OP.mult)
            sVn = cst.tile([128, NT], F32)
            v.tensor_scalar(sVn[:], zcol[:], -2.0 * math.pi, None, OP.mult)
            invz = cst.tile([128, NT], F32)
            invz_s = cst.tile([128, NT], F32)
            v.reciprocal_approx_accurate(invz[:], zcol[:], invz_s[:])

            # matmul rhs tensors: c_k * u^k
            RA_L_t = cst.tile([6, NL], F32, name="RA_L")
            v.tensor_scalar(RA_L_t[:], upowL[:], aext[:], None, OP.mult)
            RB_L_t = cst.tile([6, NL], F32, name="RB_L")
            v.tensor_scalar(RB_L_t[:], upowL[:], bext[:], None, OP.mult)
            # row 0 <- 2*ln(w_j) quadrature ln-weights
            nc.sync.dma_start(out=RB_L_t[0:1, :], in_=lnw2L_d[:, :])
            RA_V_t = cst.tile([6, NV], F32, name="RA_V")
            v.tensor_scalar(RA_V_t[:], upowV[:], aext[:], None, OP.mult)
            RAB_V_t = cst.tile([6, NV], F32, name="RAB_V")
            v.tensor_scalar(RAB_V_t[:], upowV[:], abext[:], None, OP.mult)
            RA_L, RB_L, RA_V, RAB_V = RA_L_t[:], RB_L_t[:], RA_V_t[:], RAB_V_t[:]

            # accumulators & bias columns
            accLre = cst.tile([128, NT], F32)
            accLim = cst.tile([128, NT], F32)
            accVre = cst.tile([128, NT], F32)
            accVim = cst.tile([128, NT], F32)
            nhln2 = cst.tile([128, 1], F32)    # -ln2/2 bias column
            v.memset(nhln2[:], -0.5 * LN2)
            c_one = cst.tile([128, 1], F32)
            v.memset(c_one[:], 1.0)
            c_negk = cst.tile([128, 1], F32)   # -(1-eps)
            v.memset(c_negk[:], -(1.0 - EPS))
            c_eps = cst.tile([128, 1], F32)
            v.memset(c_eps[:], EPS)
            c_onep = cst.tile([128, 1], F32)   # 1+eps
            v.memset(c_onep[:], 1.0 + EPS)

            # ---------------- main loop ----------------
            for rep in range(reps):
              for t in range(NT):
                lhs = zpow[:, t * 128: (t + 1) * 128]
                nzs4_t = nzs4c[:, t: t + 1]
                c1_t = c1c[:, t: t + 1]
                ln4_t = lnzs4[:, t: t + 1]

                # ======== L integral (N=128) ========
                N = NL
                pa_ps = pspool.tile([128, N], F32, tag="pa", name="paL")
                pb_ps = pspool.tile([128, N], F32, tag="pb", name="pbL")
                nc.tensor.matmul(pa_ps[:], lhs, RA_L, start=True, stop=True)
                nc.tensor.matmul(pb_ps[:], lhs, RB_L, start=True, stop=True)

                e_a2 = W("a0")
                sc.activation(e_a2[:], pa_ps[:], AF.Exp, bias=ln4_t, scale=1.0)
                omz4 = W("a2")
                v.tensor_scalar(omz4[:], u4Lb, nzs4_t, 1.0, OP.mult, OP.add)
                Dp = W("a3")
                v.tensor_scalar(Dp[:], u4Lb, c1_t, EPS, OP.mult, OP.add)
                X = W("a4")
                gp.tensor_tensor(X[:], omz4[:], e_a2[:], OP.mult)
                sqDp = W("a5")
                gp.tensor_tensor(sqDp[:], Dp[:], Dp[:], OP.mult)
                n2 = W("a6")
                v.tensor_scalar(n2[:], sqDp[:], EPS2, None, OP.add)
                rn2 = W("a5")
                v.reciprocal_approx_fast(rn2[:], n2[:])
                t_ = W("a6")
                v.tensor_tensor(t_[:], X[:], rn2[:], OP.mult)
                tDp = W("a4")
                v.tensor_tensor(tDp[:], t_[:], Dp[:], OP.mult)
                # re = tDp-(1-eps), im = eps*(1-t_): folded into ACT affine below
                sgn = W("a8", dt=BF16)
                sc.activation(sgn[:], t_[:], AF.Sign, bias=c_one[:, 0:1], scale=-1.0)
                sqre = W("a5")
                sc.activation(sqre[:], tDp[:], AF.Square, bias=c_negk[:, 0:1], scale=1.0)
                sqim = W("a9")
                sc.activation(sqim[:], t_[:], AF.Square, bias=c_eps[:, 0:1], scale=-EPS)
                r2s = W("a6")
                gp.tensor_tensor(r2s[:], sqre[:], sqim[:], OP.add)
                lnom = W("a7")
                sc.activation(lnom[:], omz4[:], AF.Ln)
                lnr2s = W("a5")
                sc.activation(lnr2s[:], r2s[:], AF.Ln)
                r_ = W("aA", dt=BF16)
                sc.activation(r_[:], lnr2s[:], AF.Exp, scale=0.5)
                absre = W("a2", dt=BF16)
                sc.activation(absre[:], tDp[:], AF.Abs, bias=c_negk[:, 0:1], scale=1.0)
                rlg = W("a3", dt=BF16)
                v.tensor_tensor(rlg[:], absre[:], r_[:], OP.add)
                lnrlg = W("aB")
                sc.activation(lnrlg[:], rlg[:], AF.Ln)
                base = W("a0")
                v.tensor_tensor(base[:], pb_ps[:], lnom[:], OP.subtract)
                base2 = W("a2")
                v.tensor_tensor(base2[:], base[:], lnr2s[:], OP.subtract)
                lnim2 = W("a6")
                sc.activation(lnim2[:], sqim[:], AF.Ln)
                lnglg = W("a5")
                v.tensor_tensor(lnglg[:], base2[:], lnrlg[:], OP.add)
                SS = W("a1", dt=BF16)            # -> becomes igq after swap
                sc.activation(SS[:], lnglg[:], AF.Exp, bias=nhln2[:, 0:1], scale=0.5)
                prt = W("a3")
                gp.tensor_tensor(prt[:], base2[:], lnrlg[:], OP.subtract)
                lngsm = W("a0")
                v.tensor_tensor(lngsm[:], prt[:], lnim2[:], OP.add)
                TTs = W("a7", dt=BF16)           # -> becomes igre after swap
                sc.activation(TTs[:], lngsm[:], AF.Exp, bias=nhln2[:, 0:1], scale=0.5)
                TTs2 = W("a9", dt=BF16)
                v.tensor_copy(TTs2[:], TTs[:])
                m = W("aA", dt=BF16)
                v.tensor_scalar(m[:], tDp[:], 1.0 - EPS, None, OP.is_ge)
                # igre = m ? SS : TTs ; igq = m ? TTs : SS
                v.copy_predicated(TTs[:], m[:].bitcast(U16), SS[:])
                v.copy_predicated(SS[:], m[:].bitcast(U16), TTs2[:])
                v.tensor_reduce(accLre[:, t: t + 1], TTs[:], AX.X, OP.add)
                igqs = W("a2", dt=BF16)
                v.scalar_tensor_tensor(
                    igqs[:], SS[:], 1.0, sgn[:], OP.mult, OP.mult,
                    accum_out=accLim[:, t: t + 1],
                )

                # ======== V integral (N=192) ========
                N = NV
                pa_ps = pspool.tile([128, N], F32, tag="pa", name="paV")
                pab_ps = pspool.tile([128, N], F32, tag="pb", name="pabV")
                nc.tensor.matmul(pa_ps[:], lhs, RA_V, start=True, stop=True)
                nc.tensor.matmul(pab_ps[:], lhs, RAB_V, start=True, stop=True)

                e_a2 = W("b0", N)
                sc.activation(e_a2[:], pa_ps[:], AF.Exp, bias=ln4_t, scale=1.0)
                omz4 = W("b2", N)
                v.tensor_scalar(omz4[:], u4Vb, nzs4_t, 1.0, OP.mult, OP.add)
                Y = W("b3", N)
                v.tensor_scalar(Y[:], u4Vb, c1_t, None, OP.mult)
                X = W("b4", N)
                gp.tensor_tensor(X[:], omz4[:], e_a2[:], OP.mult)
                Xp = W("b0", N)
                v.tensor_scalar(Xp[:], X[:], EPS, None, OP.add)
                sqXp = W("b2", N)
                gp.tensor_tensor(sqXp[:], Xp[:], Xp[:], OP.mult)
                n2v = W("b4", N)
                v.tensor_scalar(n2v[:], sqXp[:], EPS2, None, OP.add)
                rn2v = W("b2", N)
                v.reciprocal_approx_fast(rn2v[:], n2v[:])
                t2 = W("b4", N)
                v.tensor_tensor(t2[:], Y[:], rn2v[:], OP.mult)
                t2Xp = W("b3", N)
                v.tensor_tensor(t2Xp[:], t2[:], Xp[:], OP.mult)
                # re2 = 1+eps-t2Xp, im2 = eps*t2+eps: folded into ACT affine
                sqre2 = W("b0", N)
                sc.activation(sqre2[:], t2Xp[:], AF.Square, bias=c_onep[:, 0:1], scale=-1.0)
                sqim2 = W("b2", N)
                sc.activation(sqim2[:], t2[:], AF.Square, bias=c_eps[:, 0:1], scale=EPS)
                r2s2 = W("b4", N)
                gp.tensor_tensor(r2s2[:], sqre2[:], sqim2[:], OP.add)
                lnr2s2 = W("b0", N)
                sc.activation(lnr2s2[:], r2s2[:], AF.Ln)
                r2v = W("b5", N, dt=BF16)
                sc.activation(r2v[:], lnr2s2[:], AF.Exp, scale=0.5)
                absre2 = W("b6", N, dt=BF16)
                sc.activation(absre2[:], t2Xp[:], AF.Abs, bias=c_onep[:, 0:1], scale=-1.0)
                rlg2 = W("b1", N, dt=BF16)
                v.tensor_tensor(rlg2[:], absre2[:], r2v[:], OP.add)
                lnrlg2 = W("b5", N)
                sc.activation(lnrlg2[:], rlg2[:], AF.Ln)
                base2v = W("b2", N)
                v.tensor_tensor(base2v[:], pab_ps[:], lnr2s2[:], OP.subtract)
                lnim2v = W("b6", N)
                sc.activation(lnim2v[:], sqim2[:], AF.Ln)
                lnglg2 = W("b4", N)
                v.tensor_tensor(lnglg2[:], base2v[:], lnrlg2[:], OP.add)
                SSv = W("b0", N)                 # -> becomes M2 after swap
                sc.activation(SSv[:], lnglg2[:], AF.Exp, bias=nhln2[:, 0:1], scale=0.5)
                prt_v = W("b1", N)
                gp.tensor_tensor(prt_v[:], base2v[:], lnrlg2[:], OP.subtract)
                lngsm2 = W("b4", N)
                v.tensor_tensor(lngsm2[:], prt_v[:], lnim2v[:], OP.add)
                TTv = W("b2", N)                 # -> becomes P2 after swap
                sc.activation(TTv[:], lngsm2[:], AF.Exp, bias=nhln2[:, 0:1], scale=0.5)
                TTv2 = W("b5", N)
                v.tensor_copy(TTv2[:], TTv[:])
                m2 = W("b1", N, dt=BF16)
                v.tensor_scalar(m2[:], t2Xp[:], 1.0 + EPS, None, OP.is_le)
                # P2 = m2 ? SSv : TTv ; M2 = m2 ? TTv : SSv
                v.copy_predicated(TTv[:], m2[:].bitcast(U16), SSv[:])
                v.copy_predicated(SSv[:], m2[:].bitcast(U16), TTv2[:])
                P2 = TTv
                M2 = SSv

                zdb = W("b3", N, dt=BF16)
                v.tensor_scalar(zdb[:], u2Vb, zs2c[:, t: t + 1], EPS, OP.mult, OP.add)
                sqzd = W("b4", N)
                v.tensor_tensor(sqzd[:], zdb[:], zdb[:], OP.mult)
                ndn = W("b5", N)
                v.tensor_scalar(ndn[:], sqzd[:], EPS2, None, OP.add)
                rndr = W("b4", N)
                v.reciprocal_approx_fast(rndr[:], ndn[:])
                rnd = W("b5", N)
                gp.tensor_tensor(rnd[:], rndr[:], wVb, OP.mult)

                P2m = W("b6", N, dt=BF16)
                v.tensor_scalar(P2m[:], P2[:], -1.0, None, OP.add)
                M2b = W("b4", N, dt=BF16)
                v.tensor_copy(M2b[:], M2[:])
                A12 = W("b1", N, dt=BF16)
                v.tensor_tensor(A12[:], P2m[:], zdb[:], OP.mult)
                A4 = W("b2", N, dt=BF16)
                v.scalar_tensor_tensor(A4[:], M2b[:], -EPS, A12[:], OP.mult, OP.add)
                igre = W("b1", N)
                v.scalar_tensor_tensor(
                    igre[:], A4[:], 1.0, rnd[:], OP.mult, OP.mult,
                    accum_out=accVre[:, t: t + 1],
                )
                B1 = W("b0", N, dt=BF16)
                v.tensor_tensor(B1[:], M2b[:], zdb[:], OP.mult)
                B3 = W("b3", N, dt=BF16)
                v.scalar_tensor_tensor(B3[:], P2m[:], EPS, B1[:], OP.mult, OP.add)
                igim = W("b2", N)
                v.scalar_tensor_tensor(
                    igim[:], B3[:], 1.0, rnd[:], OP.mult, OP.mult,
                    accum_out=accVim[:, t: t + 1],
                )

            # ---------------- finals ----------------
            Lre_f = cst.tile([128, NT], F32)
            v.tensor_tensor(Lre_f[:], accLre[:], sL[:], OP.mult)
            Lim_f = cst.tile([128, NT], F32)
            v.tensor_tensor(Lim_f[:], accLim[:], sLn[:], OP.mult)
            Vraw = cst.tile([128, NT], F32)
            v.tensor_tensor(Vraw[:], accVre[:], sV[:], OP.mult)
            Vre_f = cst.tile([128, NT], F32)
            v.scalar_tensor_tensor(Vre_f[:], invz[:], -2.0 * math.pi, Vraw[:], OP.mult, OP.add)
            Vim_f = cst.tile([128, NT], F32)
            v.tensor_tensor(Vim_f[:], accVim[:], sVn[:], OP.mult)

            for row, tile in ((0, Lre_f), (1, Lim_f), (2, Vre_f), (3, Vim_f)):
                nc.sync.dma_start(
                    out=out_d[row, :].rearrange("(t p) -> p t", p=128), in_=tile[:]
                )
    return nc


_NC_CACHE = {}


def _restrict_act_tables(nc):
    """Monkeypatch table-set selection to the one set that serves every
    activation this kernel uses (exp/ln/square/sign/abs/copy/identity) so
    the steady state has zero ACT_TABLE_LOADs."""
    import types
    from concourse.hw_specs import get_activation_tables

    def _patched(self):
        # keep full list length so act_func_set_id indices stay aligned with
        # act_info.json; empty the other sets so only this one can be chosen
        tables = [(k, (v if k == "natural_log_exp_and_others" else set()))
                  for k, v in get_activation_tables(self.m.arch).items()]
        bacc._bass_rust.insert_act_table_loads(self, tables)

    nc.insert_act_table_loads = types.MethodType(_patched, nc)


def kernel(a, b, zs):
    a = np.asarray(a, dtype=np.float32)
    b = np.asarray(b, dtype=np.float32)
    zs = np.asarray(zs, dtype=np.float32)
    if "nc" not in _NC_CACHE:
        nc0 = build_nc()
        _restrict_act_tables(nc0)
        nc0.finalize()
        _NC_CACHE["nc"] = nc0
    nc = _NC_CACHE["nc"]
    in_maps = [
        {"a": a, "b": b, "zs": zs[i * BLOC: (i + 1) * BLOC].copy()}
        for i in range(NCORES)
    ]
    res = bass_utils.run_bass_kernel_spmd(nc, in_maps, core_ids=list(range(NCORES)))
    out = np.concatenate([res.results[i]["out"] for i in range(NCORES)], axis=1)
    return out.astype(np.float32)


if __name__ == "__main__":
    rng = np.random.default_rng(0)
    out = kernel(
        rng.standard_normal(5).astype(np.float32),
        rng.standard_normal(5).astype(np.float32),
        (0.02 + 0.975 * rng.random(8192)).astype(np.float32),
    )
    print(out.shape, out.dtype, out[:, :3])


# revision 19
# speedup vs baseline: 8.2628x; 1.4496x over previous
"""AdSBHNet integral kernel for 8 TRN2 NeuronCores — transposed layout.

Math (all-real reformulation of the complex reference):
  poly(c,z) = sum_{i=1..5} c_i z^i ;  f = (1-z^4) e^{poly(a,z)} ; g = e^{poly(b,z)}/(1-z^4)
  z = zs*u.
  L: w  = A/(D+eps+i*eps) - 1 + eps(1+i),  A = zs^4 f(z), D = z^4 f(zs)
     integrand = sqrt(g)/sqrt(w);  L = (2/pi) * zs * sum_j(w_j * integrand_j)
  V: inner = 1 - Y/(X+eps+i*eps) + eps(1+i), Y = z^4 f(zs), X = zs^4 f(z)
     term = sqrt(f g)/sqrt(inner) - 1; integrand = term/(z^2+eps(1+i))
     V = 2pi*zs*sum_j(w_j integrand_j) - 2pi/zs
  Complex sqrt of w=re+i*im with r=|w|: sqrt(w) = p + i*q, p=sqrt((r+re)/2),
  q=sign(im)*sqrt((r-re)/2); 1/sqrt(w) = (p-i*q)/r.  For sqrt(g)/sqrt(w):
    sqrt(g)*p/r = sqrt(G*(r+re)), sqrt(g)*|q|/r = sqrt(G*(r-re)), G = g/(2 r^2).
  r-|re| cancels catastrophically, so compute rlarge = r+|re| and
  rsmall = im^2/rlarge and route by sign(re) with predicated copies.

Quadrature: the reference's 2000/1500-pt uniform trapezoid sums are replaced
by an equivalent 128-node mixed rule per integral (validated ~6e-6 relative):
Gauss-Legendre on the smooth middle + the exact trapezoid points near the
eps-regularized singular ends (L: 64 GL + last 64 pts; V: first 48 pts +
48 GL + last 32 pts).

Layout: u-nodes live in the PARTITION dim (128 exactly), the 1024 batch rows
of each core in the FREE dim. Per-node constants are [128,1] per-partition
scalars (free in tensor_scalar/bias); per-row quantities are [1,1024] rows
broadcast once via a K=1 PE matmul. Each integral is ONE wide pass of
~30 instructions of free-size 1024. poly(a,z)+ln(zs^4) comes from a K=6
TensorE matmul (lhsT = const u-powers [6,128], rhs = a_k-scaled zs-powers
[6,1024], bias rows ride as k=0 terms). The weighted node-sum is a K=128 PE
matmul with the quadrature weights (x final scale) in lhsT.

Sharding: pure data parallel, zs batch split 8 ways; a/b replicated.
"""

import math
import sys

import numpy as np

sys.path.insert(0, "/opt/trn_rl_repo")

import concourse.bass as bass
import concourse.bacc as bacc
import concourse.mybir as mybir
from concourse import bass_utils
from concourse.tile import TileContext

F32 = mybir.dt.float32
I32 = mybir.dt.int32
BF16 = mybir.dt.bfloat16
U16 = mybir.dt.uint16
OP = mybir.AluOpType
AF = mybir.ActivationFunctionType

EPS = 1e-6
EPS2 = EPS * EPS
NU_L = 2000
NU_V = 1500
B = 8192
NCORES = 8
BLOC = B // NCORES       # 1024 rows per core
H_L = (1.0 - 2 * EPS) / (NU_L - 1)
H_V = (1.0 - 2 * EPS) / (NU_V - 1)
LN2 = math.log(2.0)
NN = 128                 # u-nodes per integral == partition count


def _mixed_nodes(Nu, h, n_head, n_gl, n_tail):
    """Nodes/weights replicating the Nu-pt trapezoid sum h*(f0/2+...+fN/2)
    with Gauss-Legendre on the smooth middle (float64)."""
    u = EPS + h * np.arange(Nu)
    nodes, wts = [], []
    if n_head:
        nodes.append(u[: n_head + 1])
        w = np.full(n_head + 1, h)
        w[0] = w[-1] = h / 2
        wts.append(w)
    lo = u[n_head]
    hi = u[Nu - 1 - n_tail]
    x, w = np.polynomial.legendre.leggauss(n_gl)
    nodes.append(0.5 * (hi + lo) + 0.5 * (hi - lo) * x)
    wts.append(0.5 * (hi - lo) * w)
    nodes.append(u[Nu - 1 - n_tail:])
    wt = np.full(n_tail + 1, h)
    wt[0] = wt[-1] = h / 2
    wts.append(wt)
    return np.concatenate(nodes), np.concatenate(wts)


_UL, _WL = _mixed_nodes(NU_L, H_L, 0, 64, 63)      # 128 nodes
_UV, _WV = _mixed_nodes(NU_V, H_V, 47, 48, 31)     # 128 nodes
assert len(_UL) == NN and len(_UV) == NN

_K6 = np.arange(6.0)
_UPOWL_T = (_UL[None, :] ** _K6[:, None]).astype(np.float32)   # [6, 128]
_UPOWV_T = (_UV[None, :] ** _K6[:, None]).astype(np.float32)   # [6, 128]
# per-node columns: u4L, -u4L, u4V, -u4V, u2V,
#                   wLre=(2/pi)wL, wLim=-(2/pi)wL, wVre=2pi wV, wVim=-2pi wV
_NCOL = np.stack(
    [
        _UL**4, -(_UL**4), _UV**4, -(_UV**4), _UV**2,
        (2.0 / math.pi) * _WL, -(2.0 / math.pi) * _WL,
        2.0 * math.pi * _WV, -2.0 * math.pi * _WV,
    ],
    axis=1,
).astype(np.float32)                                            # [128, 9]
_NCOL_BF = _NCOL[:, 5:9].astype(np.float32)  # weight cols, bf16-cast on chip


def build_nc(reps=1):
    nc = bacc.Bacc("TRN2", target_bir_lowering=False, debug=False, num_devices=NCORES)
    a_d = nc.declare_dram_parameter("a", [5], F32, isOutput=False)
    b_d = nc.declare_dram_parameter("b", [5], F32, isOutput=False)
    zs_d = nc.declare_dram_parameter("zs", [BLOC], F32, isOutput=False)
    out_d = nc.declare_dram_parameter("out", [4, BLOC], F32, isOutput=True)

    upowL_d = nc.inline_tensor(_UPOWL_T, name="upowL")
    upowV_d = nc.inline_tensor(_UPOWV_T, name="upowV")
    ncol_d = nc.inline_tensor(_NCOL, name="ncol")

    with TileContext(nc) as tc:
        with (
            tc.tile_pool(name="cst", bufs=1) as cst,
            tc.tile_pool(name="wk", bufs=1) as wk,
            tc.tile_pool(name="ps", bufs=1, space="PSUM") as pspool,
            tc.tile_pool(name="pss", bufs=1, space="PSUM") as pssetup,
        ):
            v = nc.vector
            sc = nc.scalar
            gp = nc.gpsimd

            def W(tag, dt=F32, nm=None):
                return wk.tile([NN, BLOC], dt, tag=tag, name=nm or f"t{tag}")

            def R(tag, dt=F32, nm=None):
                return wk.tile([1, BLOC], dt, tag=tag, name=nm or f"r{tag}")

            # ---------------- setup ----------------
            zrow = cst.tile([1, BLOC], F32)
            nc.sync.dma_start(out=zrow[:], in_=zs_d[:].rearrange("(o n) -> o n", o=1))
            upL = cst.tile([6, NN], F32)
            nc.sync.dma_start(out=upL[:], in_=upowL_d[:, :])
            upV = cst.tile([6, NN], F32)
            nc.sync.dma_start(out=upV[:], in_=upowV_d[:, :])
            ncol = cst.tile([NN, 9], F32)
            nc.sync.dma_start(out=ncol[:], in_=ncol_d[:, :])
            wcols = cst.tile([NN, 4], BF16)
            v.tensor_copy(wcols[:], ncol[:, 5:9])

            aext = cst.tile([6, 1], F32)
            bext = cst.tile([6, 1], F32)
            v.memset(aext[:], 0.0)
            v.memset(bext[:], 0.0)
            nc.sync.dma_start(out=aext[1:6, 0:1], in_=a_d[:])
            nc.sync.dma_start(out=bext[1:6, 0:1], in_=b_d[:])
            abext = cst.tile([6, 1], F32)
            v.tensor_tensor(abext[:], aext[:], bext[:], OP.add)

            ones6 = cst.tile([1, 6], F32)
            v.memset(ones6[:], 1.0)
            ones128 = cst.tile([1, NN], F32)
            v.memset(ones128[:], 1.0)
            kcol_i = cst.tile([6, 1], I32)
            gp.iota(kcol_i[:], pattern=[[1, 1]], base=0, channel_multiplier=1)
            kcol6 = cst.tile([6, 1], F32)
            v.tensor_copy(kcol6[:], kcol_i[:])

            lnz = cst.tile([1, BLOC], F32)
            sc.activation(lnz[:], zrow[:], AF.Ln)

            # zpow [6, BLOC]: row k = zs^k via exp(k ln zs)
            klnz = wk.tile([6, BLOC], F32, tag="s0", name="klnz")
            for c0 in range(0, BLOC, 512):
                ps6 = pssetup.tile([6, 512], F32, tag="pd", name=f"ps6_{c0}")
                nc.tensor.matmul(ps6[:], ones6[:], lnz[:, c0:c0 + 512],
                                 start=True, stop=True)
                v.tensor_scalar(klnz[:, c0:c0 + 512], ps6[:], kcol6[:], None, OP.mult)
            zpow = cst.tile([6, BLOC], F32)
            sc.activation(zpow[:], klnz[:], AF.Exp)

            # matmul rhs tensors [6, BLOC]
            lnzs4row = cst.tile([1, BLOC], F32)
            v.tensor_scalar(lnzs4row[:], lnz[:], 4.0, None, OP.mult)
            rhs_pa = cst.tile([6, BLOC], F32)
            v.tensor_scalar(rhs_pa[:], zpow[:], aext[:], None, OP.mult)
            v.tensor_copy(rhs_pa[0:1, :], lnzs4row[:])
            rhs_pb = cst.tile([6, BLOC], F32)
            v.tensor_scalar(rhs_pb[:], zpow[:], bext[:], None, OP.mult)
            v.tensor_scalar(rhs_pb[0:1, :], lnz[:], 2.0, None, OP.mult)
            rhs_pab = cst.tile([6, BLOC], F32)
            v.tensor_scalar(rhs_pab[:], zpow[:], abext[:], None, OP.mult)

            # per-row quantities [1, BLOC]
            zs2row = cst.tile([1, BLOC], F32)
            v.tensor_tensor(zs2row[:], zrow[:], zrow[:], OP.mult)
            zs4row = cst.tile([1, BLOC], F32)
            v.tensor_tensor(zs4row[:], zs2row[:], zs2row[:], OP.mult)
            # pa(zs): K=6 matmul -> [1, BLOC]
            e_pazrow = cst.tile([1, BLOC], F32)
            for c0 in range(0, BLOC, 512):
                pz = pssetup.tile([1, 512], F32, tag="pd", name=f"pz_{c0}")
                nc.tensor.matmul(pz[:], aext[:], zpow[:, c0:c0 + 512],
                                 start=True, stop=True)
                sc.activation(e_pazrow[:, c0:c0 + 512], pz[:], AF.Exp)
            omzs4row = R("s1", nm="omzs4row")
            v.tensor_scalar(omzs4row[:], zs4row[:], -1.0, 1.0, OP.mult, OP.add)
            fzsrow = R("s2", nm="fzsrow")
            v.tensor_tensor(fzsrow[:], e_pazrow[:], omzs4row[:], OP.mult)
            c1row = cst.tile([1, BLOC], F32)
            v.tensor_tensor(c1row[:], zs4row[:], fzsrow[:], OP.mult)
            invzrow = cst.tile([1, BLOC], F32)
            sc.activation(invzrow[:], lnz[:], AF.Exp, scale=-1.0)

            # broadcasts [128, BLOC] via K=1 matmul
            zs2b = cst.tile([NN, BLOC], F32)
            c1b = cst.tile([NN, BLOC], F32)
            for row, dst in ((zs2row, zs2b), (c1row, c1b)):
                for c0 in range(0, BLOC, 512):
                    pb_ = pssetup.tile([NN, 512], F32, tag="pe", name=f"bc_{c0}")
                    nc.tensor.matmul(pb_[:], ones128[:], row[:, c0:c0 + 512],
                                     start=True, stop=True)
                    v.tensor_copy(dst[:, c0:c0 + 512], pb_[:])
            zs4b = cst.tile([NN, BLOC], F32)
            gp.tensor_tensor(zs4b[:], zs2b[:], zs2b[:], OP.mult)

            # node-constant columns
            u4L_c = ncol[:, 0:1]
            nu4L_c = ncol[:, 1:2]
            u4V_c = ncol[:, 2:3]
            nu4V_c = ncol[:, 3:4]
            u2V_c = ncol[:, 4:5]
            wLre_c = wcols[:, 0:1]
            wLim_c = wcols[:, 1:2]
            wVre_c = wcols[:, 2:3]
            wVim_c = wcols[:, 3:4]

            nhln2 = cst.tile([NN, 1], F32)
            v.memset(nhln2[:], -0.5 * LN2)
            c_one = cst.tile([NN, 1], F32)
            v.memset(c_one[:], 1.0)
            c_negk = cst.tile([NN, 1], F32)
            v.memset(c_negk[:], -(1.0 - EPS))
            c_eps = cst.tile([NN, 1], F32)
            v.memset(c_eps[:], EPS)
            c_onep = cst.tile([NN, 1], F32)
            v.memset(c_onep[:], 1.0 + EPS)

            def reduce_sum(wcol, rhs, nm):
                red = pspool.tile([1, BLOC], F32, tag="pr", name=nm)
                for c0 in range(0, BLOC, 512):
                    nc.tensor.matmul(red[0:1, c0:c0 + 512], wcol,
                                     rhs[:, c0:c0 + 512], start=True, stop=True)
                return red

            # ================ L pass ================
            pa_ps = pspool.tile([NN, BLOC], F32, tag="pa", name="paL")
            pb_ps = pspool.tile([NN, BLOC], F32, tag="pb", name="pbL")
            for c0 in range(0, BLOC, 512):
                nc.tensor.matmul(pa_ps[:, c0:c0 + 512], upL[:], rhs_pa[:, c0:c0 + 512],
                                 start=True, stop=True)
            for c0 in range(0, BLOC, 512):
                nc.tensor.matmul(pb_ps[:, c0:c0 + 512], upL[:], rhs_pb[:, c0:c0 + 512],
                                 start=True, stop=True)

            e_a2 = W("a0")
            sc.activation(e_a2[:], pa_ps[:], AF.Exp)
            omz4 = W("a2")
            v.tensor_scalar(omz4[:], zs4b[:], nu4L_c, 1.0, OP.mult, OP.add)
            Dp = W("a3")
            v.tensor_scalar(Dp[:], c1b[:], u4L_c, EPS, OP.mult, OP.add)
            X = W("a4")
            gp.tensor_tensor(X[:], omz4[:], e_a2[:], OP.mult)
            sqDp = W("a5")
            gp.tensor_tensor(sqDp[:], Dp[:], Dp[:], OP.mult)
            n2 = W("a6")
            v.tensor_scalar(n2[:], sqDp[:], EPS2, None, OP.add)
            rn2 = W("a5")
            v.reciprocal_approx_fast(rn2[:], n2[:])
            t_ = W("a6")
            v.tensor_tensor(t_[:], X[:], rn2[:], OP.mult)
            tDp = W("a4")
            v.tensor_tensor(tDp[:], t_[:], Dp[:], OP.mult)
            sgn = W("a8", dt=BF16)
            sc.activation(sgn[:], t_[:], AF.Sign, bias=c_one[:, 0:1], scale=-1.0)
            sqre = W("a5")
            sc.activation(sqre[:], tDp[:], AF.Square, bias=c_negk[:, 0:1], scale=1.0)
            sqim = W("a9")
            sc.activation(sqim[:], t_[:], AF.Square, bias=c_eps[:, 0:1], scale=-EPS)
            r2s = W("a6")
            gp.tensor_tensor(r2s[:], sqre[:], sqim[:], OP.add)
            lnom = W("a7")
            sc.activation(lnom[:], omz4[:], AF.Ln)
            lnr2s = W("a5")
            sc.activation(lnr2s[:], r2s[:], AF.Ln)
            r_ = W("aA", dt=BF16)
            sc.activation(r_[:], lnr2s[:], AF.Exp, scale=0.5)
            absre = W("a2", dt=BF16)
            sc.activation(absre[:], tDp[:], AF.Abs, bias=c_negk[:, 0:1], scale=1.0)
            rlg = W("a3", dt=BF16)
            v.tensor_tensor(rlg[:], absre[:], r_[:], OP.add)
            lnrlg = W("aB")
            sc.activation(lnrlg[:], rlg[:], AF.Ln)
            base = W("a0")
            v.tensor_tensor(base[:], pb_ps[:], lnom[:], OP.subtract)
            base2 = W("a2")
            v.tensor_tensor(base2[:], base[:], lnr2s[:], OP.subtract)
            lnim2 = W("a6")
            sc.activation(lnim2[:], sqim[:], AF.Ln)
            lnglg = W("a5")
            v.tensor_tensor(lnglg[:], base2[:], lnrlg[:], OP.add)
            SS = W("a1", dt=BF16)            # -> becomes igq after swap
            sc.activation(SS[:], lnglg[:], AF.Exp, bias=nhln2[:, 0:1], scale=0.5)
            prt = W("a3")
            gp.tensor_tensor(prt[:], base2[:], lnrlg[:], OP.subtract)
            lngsm = W("a0")
            v.tensor_tensor(lngsm[:], prt[:], lnim2[:], OP.add)
            TTs = W("a7", dt=BF16)           # -> becomes igre after swap
            sc.activation(TTs[:], lngsm[:], AF.Exp, bias=nhln2[:, 0:1], scale=0.5)
            TTs2 = W("a9", dt=BF16)
            v.tensor_copy(TTs2[:], TTs[:])
            m = W("aA", dt=BF16)
            v.tensor_scalar(m[:], tDp[:], 1.0 - EPS, None, OP.is_ge)
            v.copy_predicated(TTs[:], m[:].bitcast(U16), SS[:])
            v.copy_predicated(SS[:], m[:].bitcast(U16), TTs2[:])
            igqs = W("a2", dt=BF16)
            v.tensor_tensor(igqs[:], SS[:], sgn[:], OP.mult)
            redLre = reduce_sum(wLre_c, TTs, "redLre")
            redLim = reduce_sum(wLim_c, igqs, "redLim")
            outLre = cst.tile([1, BLOC], F32)
            sc.activation(outLre[:], redLre[0:1, :], AF.Copy)
            nc.sync.dma_start(out=out_d[0, :].rearrange("(o n) -> o n", o=1),
                              in_=outLre[:])
            outLim = cst.tile([1, BLOC], F32)
            sc.activation(outLim[:], redLim[0:1, :], AF.Copy)
            nc.sync.dma_start(out=out_d[1, :].rearrange("(o n) -> o n", o=1),
                              in_=outLim[:])

            # ================ V pass ================
            pa2_ps = pspool.tile([NN, BLOC], F32, tag="pa", name="paV")
            pab_ps = pspool.tile([NN, BLOC], F32, tag="pb", name="pabV")
            for c0 in range(0, BLOC, 512):
                nc.tensor.matmul(pa2_ps[:, c0:c0 + 512], upV[:], rhs_pa[:, c0:c0 + 512],
                                 start=True, stop=True)
            for c0 in range(0, BLOC, 512):
                nc.tensor.matmul(pab_ps[:, c0:c0 + 512], upV[:],
                                 rhs_pab[:, c0:c0 + 512], start=True, stop=True)

            e_a2v = W("b0")
            sc.activation(e_a2v[:], pa2_ps[:], AF.Exp)
            omz4v = W("b2")
            v.tensor_scalar(omz4v[:], zs4b[:], nu4V_c, 1.0, OP.mult, OP.add)
            Y = W("b3")
            v.tensor_scalar(Y[:], c1b[:], u4V_c, None, OP.mult)
            Xv = W("b4")
            gp.tensor_tensor(Xv[:], omz4v[:], e_a2v[:], OP.mult)
            Xp = W("b0")
            v.tensor_scalar(Xp[:], Xv[:], EPS, None, OP.add)
            sqXp = W("b2")
            gp.tensor_tensor(sqXp[:], Xp[:], Xp[:], OP.mult)
            n2v = W("b4")
            v.tensor_scalar(n2v[:], sqXp[:], EPS2, None, OP.add)
            rn2v = W("b2")
            v.reciprocal_approx_fast(rn2v[:], n2v[:])
            t2 = W("b4")
            v.tensor_tensor(t2[:], Y[:], rn2v[:], OP.mult)
            t2Xp = W("b3")
            v.tensor_tensor(t2Xp[:], t2[:], Xp[:], OP.mult)
            sqre2 = W("b0")
            sc.activation(sqre2[:], t2Xp[:], AF.Square, bias=c_onep[:, 0:1], scale=-1.0)
            sqim2 = W("b2")
            sc.activation(sqim2[:], t2[:], AF.Square, bias=c_eps[:, 0:1], scale=EPS)
            r2s2 = W("b4")
            gp.tensor_tensor(r2s2[:], sqre2[:], sqim2[:], OP.add)
            lnr2s2 = W("b0")
            sc.activation(lnr2s2[:], r2s2[:], AF.Ln)
            r2v = W("b5", dt=BF16)
            sc.activation(r2v[:], lnr2s2[:], AF.Exp, scale=0.5)
            absre2 = W("b6", dt=BF16)
            sc.activation(absre2[:], t2Xp[:], AF.Abs, bias=c_onep[:, 0:1], scale=-1.0)
            rlg2 = W("b1", dt=BF16)
            v.tensor_tensor(rlg2[:], absre2[:], r2v[:], OP.add)
            lnrlg2 = W("b5")
            sc.activation(lnrlg2[:], rlg2[:], AF.Ln)
            base2v = W("b2")
            v.tensor_tensor(base2v[:], pab_ps[:], lnr2s2[:], OP.subtract)
            lnim2v = W("b6")
            sc.activation(lnim2v[:], sqim2[:], AF.Ln)
            lnglg2 = W("b4")
            v.tensor_tensor(lnglg2[:], base2v[:], lnrlg2[:], OP.add)
            SSv = W("b0")                    # -> becomes M2 after swap
            sc.activation(SSv[:], lnglg2[:], AF.Exp, bias=nhln2[:, 0:1], scale=0.5)
            prt_v = W("b1")
            gp.tensor_tensor(prt_v[:], base2v[:], lnrlg2[:], OP.subtract)
            lngsm2 = W("b4")
            v.tensor_tensor(lngsm2[:], prt_v[:], lnim2v[:], OP.add)
            TTv = W("b2")                    # -> becomes P2 after swap
            sc.activation(TTv[:], lngsm2[:], AF.Exp, bias=nhln2[:, 0:1], scale=0.5)
            TTv2 = W("b5")
            v.tensor_copy(TTv2[:], TTv[:])
            m2 = W("b1", dt=BF16)
            v.tensor_scalar(m2[:], t2Xp[:], 1.0 + EPS, None, OP.is_le)
            v.copy_predicated(TTv[:], m2[:].bitcast(U16), SSv[:])
            v.copy_predicated(SSv[:], m2[:].bitcast(U16), TTv2[:])
            P2 = TTv
            M2 = SSv

            zdb = W("b3", dt=BF16)
            v.tensor_scalar(zdb[:], zs2b[:], u2V_c, EPS, OP.mult, OP.add)
            sqzd = W("b4")
            v.tensor_tensor(sqzd[:], zdb[:], zdb[:], OP.mult)
            ndn = W("b5")
            v.tensor_scalar(ndn[:], sqzd[:], EPS2, None, OP.add)
            rndr = W("b4")
            v.reciprocal_approx_fast(rndr[:], ndn[:])

            P2m = W("b6", dt=BF16)
            v.tensor_scalar(P2m[:], P2[:], -1.0, None, OP.add)
            M2b = W("b5", dt=BF16)
            v.tensor_copy(M2b[:], M2[:])
            A12 = W("b1", dt=BF16)
            v.tensor_tensor(A12[:], P2m[:], zdb[:], OP.mult)
            A4 = W("b2", dt=BF16)
            v.scalar_tensor_tensor(A4[:], M2b[:], -EPS, A12[:], OP.mult, OP.add)
            igre = W("b1", dt=BF16)
            v.tensor_tensor(igre[:], A4[:], rndr[:], OP.mult)
            B1 = W("b0", dt=BF16)
            v.tensor_tensor(B1[:], M2b[:], zdb[:], OP.mult)
            B3 = W("b3", dt=BF16)
            v.scalar_tensor_tensor(B3[:], P2m[:], EPS, B1[:], OP.mult, OP.add)
            igim = W("b2", dt=BF16)
            v.tensor_tensor(igim[:], B3[:], rndr[:], OP.mult)
            redVre = reduce_sum(wVre_c, igre, "redVre")
            redVim = reduce_sum(wVim_c, igim, "redVim")

            # ---------------- finals ----------------
            Vr1 = R("f0", nm="Vr1")
            v.tensor_tensor(Vr1[:], redVre[0:1, :], zrow[:], OP.mult)
            outVre = cst.tile([1, BLOC], F32)
            v.scalar_tensor_tensor(outVre[:], invzrow[:], -2.0 * math.pi, Vr1[:],
                                   OP.mult, OP.add)
            nc.sync.dma_start(out=out_d[2, :].rearrange("(o n) -> o n", o=1),
                              in_=outVre[:])
            outVim = cst.tile([1, BLOC], F32)
            v.tensor_tensor(outVim[:], redVim[0:1, :], zrow[:], OP.mult)
            nc.sync.dma_start(out=out_d[3, :].rearrange("(o n) -> o n", o=1),
                              in_=outVim[:])
    return nc


_NC_CACHE = {}


def _restrict_act_tables(nc):
    """Monkeypatch table-set selection to the one set that serves every
    activation this kernel uses (exp/ln/square/sign/abs/copy/identity) so
    the steady state has zero ACT_TABLE_LOADs."""
    import types
    from concourse.hw_specs import get_activation_tables

    def _patched(self):
        tables = [(k, (v if k == "natural_log_exp_and_others" else set()))
                  for k, v in get_activation_tables(self.m.arch).items()]
        bacc._bass_rust.insert_act_table_loads(self, tables)

    nc.insert_act_table_loads = types.MethodType(_patched, nc)


def kernel(a, b, zs):
    a = np.asarray(a, dtype=np.float32)
    b = np.asarray(b, dtype=np.float32)
    zs = np.asarray(zs, dtype=np.float32)
    if "nc" not in _NC_CACHE:
        nc0 = build_nc()
        _restrict_act_tables(nc0)
        nc0.finalize()
        _NC_CACHE["nc"] = nc0
    nc = _NC_CACHE["nc"]
    in_maps = [
        {"a": a, "b": b, "zs": zs[i * BLOC: (i + 1) * BLOC].copy()}
        for i in range(NCORES)
    ]
    res = bass_utils.run_bass_kernel_spmd(nc, in_maps, core_ids=list(range(NCORES)))
    out = np.concatenate([res.results[i]["out"] for i in range(NCORES)], axis=1)
    return out.astype(np.float32)


if __name__ == "__main__":
    rng = np.random.default_rng(0)
    out = kernel(
        rng.standard_normal(5).astype(np.float32),
        rng.standard_normal(5).astype(np.float32),
        (0.02 + 0.975 * rng.random(8192)).astype(np.float32),
    )
    print(out.shape, out.dtype, out[:, :3])


# revision 27
# speedup vs baseline: 9.8675x; 1.1942x over previous
"""AdSBHNet integral kernel for 8 TRN2 NeuronCores — transposed layout.

Math (all-real reformulation of the complex reference):
  poly(c,z) = sum_{i=1..5} c_i z^i ;  f = (1-z^4) e^{poly(a,z)} ; g = e^{poly(b,z)}/(1-z^4)
  z = zs*u.
  L: w  = A/(D+eps+i*eps) - 1 + eps(1+i),  A = zs^4 f(z), D = z^4 f(zs)
     integrand = sqrt(g)/sqrt(w);  L = (2/pi) * zs * sum_j(w_j * integrand_j)
  V: inner = 1 - Y/(X+eps+i*eps) + eps(1+i), Y = z^4 f(zs), X = zs^4 f(z)
     term = sqrt(f g)/sqrt(inner) - 1; integrand = term/(z^2+eps(1+i))
     V = 2pi*zs*sum_j(w_j integrand_j) - 2pi/zs
  Complex sqrt of w=re+i*im with r=|w|: sqrt(w) = p + i*q, p=sqrt((r+re)/2),
  q=sign(im)*sqrt((r-re)/2); 1/sqrt(w) = (p-i*q)/r.  For sqrt(g)/sqrt(w):
    sqrt(g)*p/r = sqrt(G*(r+re)), sqrt(g)*|q|/r = sqrt(G*(r-re)), G = g/(2 r^2).
  r-|re| cancels catastrophically, so compute rlarge = r+|re| and
  rsmall = im^2/rlarge and route by sign(re) with predicated copies.

Quadrature: the reference's 2000/1500-pt uniform trapezoid sums are replaced
by an equivalent 128-node mixed rule per integral (validated ~6e-6 relative):
Gauss-Legendre on the smooth middle + the exact trapezoid points near the
eps-regularized singular ends (L: 64 GL + last 64 pts; V: first 48 pts +
48 GL + last 32 pts).

Layout: u-nodes live in the PARTITION dim (128 exactly), the 1024 batch rows
of each core in the FREE dim. Per-node constants are [128,1] per-partition
scalars (free in tensor_scalar/bias); per-row quantities are [1,1024] rows
broadcast once via a K=1 PE matmul. Each integral is ONE wide pass of
~30 instructions of free-size 1024. poly(a,z)+ln(zs^4) comes from a K=6
TensorE matmul (lhsT = const u-powers [6,128], rhs = a_k-scaled zs-powers
[6,1024], bias rows ride as k=0 terms). The weighted node-sum is a K=128 PE
matmul with the quadrature weights (x final scale) in lhsT.

Sharding: pure data parallel, zs batch split 8 ways; a/b replicated.
"""

import math
import sys

import numpy as np

sys.path.insert(0, "/opt/trn_rl_repo")

import concourse.bass as bass
import concourse.bacc as bacc
import concourse.mybir as mybir
from concourse import bass_utils
from concourse.tile import TileContext

F32 = mybir.dt.float32
I32 = mybir.dt.int32
BF16 = mybir.dt.bfloat16
U16 = mybir.dt.uint16
OP = mybir.AluOpType
AF = mybir.ActivationFunctionType

EPS = 1e-6
EPS2 = EPS * EPS
NU_L = 2000
NU_V = 1500
B = 8192
NCORES = 8
BLOC = B // NCORES       # 1024 rows per core
H_L = (1.0 - 2 * EPS) / (NU_L - 1)
H_V = (1.0 - 2 * EPS) / (NU_V - 1)
LN2 = math.log(2.0)
NN = 128                 # u-nodes per integral == partition count


def _mixed_nodes(Nu, h, n_head, n_gl, n_tail):
    """Nodes/weights replicating the Nu-pt trapezoid sum h*(f0/2+...+fN/2)
    with Gauss-Legendre on the smooth middle (float64)."""
    u = EPS + h * np.arange(Nu)
    nodes, wts = [], []
    if n_head:
        nodes.append(u[: n_head + 1])
        w = np.full(n_head + 1, h)
        w[0] = w[-1] = h / 2
        wts.append(w)
    lo = u[n_head]
    hi = u[Nu - 1 - n_tail]
    x, w = np.polynomial.legendre.leggauss(n_gl)
    nodes.append(0.5 * (hi + lo) + 0.5 * (hi - lo) * x)
    wts.append(0.5 * (hi - lo) * w)
    nodes.append(u[Nu - 1 - n_tail:])
    wt = np.full(n_tail + 1, h)
    wt[0] = wt[-1] = h / 2
    wts.append(wt)
    return np.concatenate(nodes), np.concatenate(wts)


_UL, _WL = _mixed_nodes(NU_L, H_L, 0, 64, 63)      # 128 nodes
_UV, _WV = _mixed_nodes(NU_V, H_V, 47, 48, 31)     # 128 nodes
assert len(_UL) == NN and len(_UV) == NN

_K6 = np.arange(6.0)
_UPOWL_T = (_UL[None, :] ** _K6[:, None]).astype(np.float32)   # [6, 128]
_UPOWV_T = (_UV[None, :] ** _K6[:, None]).astype(np.float32)   # [6, 128]
# per-node columns: u4L, -u4L, u4V, -u4V, u2V,
#                   wLre=(2/pi)wL, wLim=-(2/pi)wL, wVre=2pi wV, wVim=-2pi wV
_NCOL = np.stack(
    [
        _UL**4, -(_UL**4), _UV**4, -(_UV**4), _UV**2,
        (2.0 / math.pi) * _WL, -(2.0 / math.pi) * _WL,
        2.0 * math.pi * _WV, -2.0 * math.pi * _WV,
    ],
    axis=1,
).astype(np.float32)                                            # [128, 9]
_NCOL_BF = _NCOL[:, 5:9].astype(np.float32)  # weight cols, bf16-cast on chip

# ---- custom DVE ops (registered into concourse.dve_ops at import) ---------
import concourse.dve_ops as _dops
from concourse.dve_spec import C0 as _C0
from concourse.dve_spec import C1 as _C1
from concourse.dve_spec import C2 as _C2
from concourse.dve_spec import Spec as _Spec
from concourse.dve_spec import Src0 as _Src0
from concourse.dve_spec import Src1 as _Src1
from concourse.dve_spec import _has_src1 as _hs1
from concourse.dve_spec import lower as _dve_lower
from concourse.dve_spec import sq as _sq
from concourse.dve_uop import DveOpSpec as _DveOpSpec


def _register_dve(name, spec):
    for op in _dops.OPS:
        if op.name == name:
            return op
    row = _dops._CUSTOM_DVE_ROW_BASE + len(_dops.OPS)
    assert row < 0x20
    _dops._SUB_OPCODE_FOR_NAME[name] = row
    shas = {}
    for ver in ("v3", "v4"):
        tmp = _DveOpSpec(name=name, opcode=row, uops=_dve_lower(spec, ver=ver),
                         rd1_en=_hs1(spec))
        shas[ver] = tmp.sha(ver)
    op = _dops.DveOp(name, spec, subdim=False, uops_sha=shas)
    _dops.OPS.append(op)
    return op


# out = (in0*s0 + s1)^2 + imm2   (n2 = (c1*u4+eps)^2+eps^2 etc.)
_AFFSQ = _register_dve("ANT_AFFSQ", _Spec(
    body=_sq(_Src0 * _C0 + _C1) + _C2,
    reference=lambda in0, in1, s0, s1, imm2: (in0 * s0 + s1) ** 2 + imm2,
))
# out = in0 * (in1*s0 + s1)      (tDp = t*(c1*u4+eps) etc.)
_MULAFF = _register_dve("ANT_MULAFF", _Spec(
    body=_Src0 * (_Src1 * _C0 + _C1),
    reference=lambda in0, in1, s0, s1, imm2: in0 * (in1 * s0 + s1),
))
from concourse.dve_spec import Zero as _Zero
from concourse.dve_spec import maxx as _maxx
from concourse.dve_spec import select as _select

# out = (in0-s0)^2 + in1        (r2s = re^2 + im^2)
_SQD_ADD = _register_dve("ANT_SQDADD", _Spec(
    body=_sq(_Src0 - _C0) + _Src1,
    reference=lambda in0, in1, s0, s1, imm2: (in0 - s0) ** 2 + in1,
))
from concourse.dve_spec import One as _One1

# out = (in0-s0)^2 + imm2*(1-in1)^2   (r2 = re^2 + im^2 from tDp,t)
_R2FULL = _register_dve("ANT_R2FULL", _Spec(
    body=_sq(_Src0 - _C0) + _sq(_One1 - _Src1) * _C2,
    reference=lambda in0, in1, s0, s1, imm2: (in0 - s0) ** 2 + imm2 * (1.0 - in1) ** 2,
))
# out = (in0-s0)^2 + imm2*(1+in1)^2   (V variant: im2 = eps^2 (1+t2)^2)
_R2FULLP = _register_dve("ANT_R2FULLP", _Spec(
    body=_sq(_Src0 - _C0) + _sq(_One1 + _Src1) * _C2,
    reference=lambda in0, in1, s0, s1, imm2: (in0 - s0) ** 2 + imm2 * (1.0 + in1) ** 2,
))
# out = |in0-s0| + in1          (rlg = |re| + r)
_ABSD_ADD = _register_dve("ANT_ABSDADD", _Spec(
    body=_maxx(_Src0 - _C0, _C0 - _Src0) + _Src1,
    reference=lambda in0, in1, s0, s1, imm2: np.abs(in0 - s0) + in1,
))
# out = in0 * sign-ish(s0 - in1): +in0 where in1 <= s0 else -in0
_SGN_LE = _register_dve("ANT_SGNLE", _Spec(
    body=_select(_Src1 <= _C0, _Src0, _Zero - _Src0),
    reference=lambda in0, in1, s0, s1, imm2: np.where(in1 <= s0, in0, -in0),
))
from concourse.dve_spec import One as _One

# out = in1*(1-in1)*in0          (c1 = zs4*(1-zs4)*e^pa(zs))
_C1ROW = _register_dve("ANT_C1ROW", _Spec(
    body=_Src1 * (_One - _Src1) * _Src0,
    reference=lambda in0, in1, s0, s1, imm2: in1 * (1.0 - in1) * in0,
))


def build_nc(reps=1):
    nc = bacc.Bacc("TRN2", target_bir_lowering=False, debug=False, num_devices=NCORES)
    a_d = nc.declare_dram_parameter("a", [5], F32, isOutput=False)
    b_d = nc.declare_dram_parameter("b", [5], F32, isOutput=False)
    zs_d = nc.declare_dram_parameter("zs", [BLOC], F32, isOutput=False)
    out_d = nc.declare_dram_parameter("out", [4, BLOC], F32, isOutput=True)

    upowL_d = nc.inline_tensor(_UPOWL_T, name="upowL")
    upowV_d = nc.inline_tensor(_UPOWV_T, name="upowV")
    ncol_d = nc.inline_tensor(_NCOL, name="ncol")

    with TileContext(nc) as tc:
        with (
            tc.tile_pool(name="cst", bufs=1) as cst,
            tc.tile_pool(name="wk", bufs=1) as wk,
            tc.tile_pool(name="ps", bufs=1, space="PSUM") as pspool,
            tc.tile_pool(name="pss", bufs=1, space="PSUM") as pssetup,
        ):
            v = nc.vector
            sc = nc.scalar
            gp = nc.gpsimd

            def W(tag, dt=F32, nm=None):
                return wk.tile([NN, BLOC], dt, tag=tag, name=nm or f"t{tag}")

            def R(tag, dt=F32, nm=None):
                return wk.tile([1, BLOC], dt, tag=tag, name=nm or f"r{tag}")

            # ---------------- setup ----------------
            zrow = cst.tile([1, BLOC], F32)
            nc.sync.dma_start(out=zrow[:], in_=zs_d[:].rearrange("(o n) -> o n", o=1))
            upL = cst.tile([6, NN], F32)
            nc.sync.dma_start(out=upL[:], in_=upowL_d[:, :])
            upV = cst.tile([6, NN], F32)
            nc.sync.dma_start(out=upV[:], in_=upowV_d[:, :])
            ncol = cst.tile([NN, 9], F32)
            nc.sync.dma_start(out=ncol[:], in_=ncol_d[:, :])
            wcols = cst.tile([NN, 4], BF16)
            v.tensor_copy(wcols[:], ncol[:, 5:9])

            aext = cst.tile([6, 1], F32)
            bext = cst.tile([6, 1], F32)
            v.memset(aext[:], 0.0)
            v.memset(bext[:], 0.0)
            nc.sync.dma_start(out=aext[1:6, 0:1], in_=a_d[:])
            nc.sync.dma_start(out=bext[1:6, 0:1], in_=b_d[:])
            abext = cst.tile([6, 1], F32)
            v.tensor_tensor(abext[:], aext[:], bext[:], OP.add)

            ones6 = cst.tile([1, 6], F32)
            v.memset(ones6[:], 1.0)
            ones128 = cst.tile([1, NN], F32)
            v.memset(ones128[:], 1.0)
            kcol_i = cst.tile([6, 1], I32)
            gp.iota(kcol_i[:], pattern=[[1, 1]], base=0, channel_multiplier=1)
            kcol6 = cst.tile([6, 1], F32)
            v.tensor_copy(kcol6[:], kcol_i[:])

            lnz = cst.tile([1, BLOC], F32)
            sc.activation(lnz[:], zrow[:], AF.Ln)

            # zpow [6, BLOC]: row k = zs^k via exp(k ln zs)
            klnz = wk.tile([6, BLOC], F32, tag="s0", name="klnz")
            for c0 in range(0, BLOC, 512):
                ps6 = pssetup.tile([6, 512], F32, tag="pd", name=f"ps6_{c0}")
                nc.tensor.matmul(ps6[:], ones6[:], lnz[:, c0:c0 + 512],
                                 start=True, stop=True)
                v.tensor_scalar(klnz[:, c0:c0 + 512], ps6[:], kcol6[:], None, OP.mult)
            zpow = cst.tile([6, BLOC], F32)
            sc.activation(zpow[:], klnz[:], AF.Exp)

            # matmul rhs tensors [6, BLOC]
            lnzs4row = cst.tile([1, BLOC], F32)
            gp.tensor_scalar(lnzs4row[:], lnz[:], 4.0, None, OP.mult)
            rhs_pa = cst.tile([6, BLOC], F32)
            gp.tensor_scalar(rhs_pa[:], zpow[:], aext[:], None, OP.mult)
            sc.activation(rhs_pa[0:1, :], lnzs4row[:], AF.Copy)
            rhs_pb = cst.tile([6, BLOC], F32)
            gp.tensor_scalar(rhs_pb[:], zpow[:], bext[:], None, OP.mult)
            sc.activation(rhs_pb[0:1, :], lnz[:], AF.Copy, scale=2.0)
            rhs_pab = cst.tile([6, BLOC], F32)
            gp.tensor_scalar(rhs_pab[:], zpow[:], abext[:], None, OP.mult)

            # per-row quantities [1, BLOC]
            zs2row = cst.tile([1, BLOC], F32)
            v.tensor_tensor(zs2row[:], zrow[:], zrow[:], OP.mult)
            zs4row = cst.tile([1, BLOC], F32)
            v.tensor_tensor(zs4row[:], zs2row[:], zs2row[:], OP.mult)
            # pa(zs): K=6 matmul -> [1, BLOC]
            e_pazrow = cst.tile([1, BLOC], F32)
            for c0 in range(0, BLOC, 512):
                pz = pssetup.tile([1, 512], F32, tag="pd", name=f"pz_{c0}")
                nc.tensor.matmul(pz[:], aext[:], zpow[:, c0:c0 + 512],
                                 start=True, stop=True)
                sc.activation(e_pazrow[:, c0:c0 + 512], pz[:], AF.Exp)
            omzs4row = R("s1", nm="omzs4row")
            gp.tensor_scalar(omzs4row[:], zs4row[:], -1.0, 1.0, OP.mult, OP.add)
            fzsrow = R("s2", nm="fzsrow")
            gp.tensor_tensor(fzsrow[:], e_pazrow[:], omzs4row[:], OP.mult)
            c1row = cst.tile([1, BLOC], F32)
            v.tensor_tensor(c1row[:], zs4row[:], fzsrow[:], OP.mult)
            invzrow = cst.tile([1, BLOC], F32)
            sc.activation(invzrow[:], lnz[:], AF.Exp, scale=-1.0)

            # broadcasts [128, BLOC] via K=1 matmul
            zs2b = cst.tile([NN, BLOC], F32)
            c1b = cst.tile([NN, BLOC], F32)
            for row, dst in ((zs2row, zs2b), (c1row, c1b)):
                for c0 in range(0, BLOC, 512):
                    pb_ = pssetup.tile([NN, 512], F32, tag="pe", name=f"bc_{c0}")
                    nc.tensor.matmul(pb_[:], ones128[:], row[:, c0:c0 + 512],
                                     start=True, stop=True)
                    sc.activation(dst[:, c0:c0 + 512], pb_[:], AF.Copy)
            zs4b = cst.tile([NN, BLOC], F32)
            gp.tensor_tensor(zs4b[:], zs2b[:], zs2b[:], OP.mult)

            # node-constant columns
            u4L_c = ncol[:, 0:1]
            nu4L_c = ncol[:, 1:2]
            u4V_c = ncol[:, 2:3]
            nu4V_c = ncol[:, 3:4]
            u2V_c = ncol[:, 4:5]
            wLre_c = wcols[:, 0:1]
            wLim_c = wcols[:, 1:2]
            wVre_c = wcols[:, 2:3]
            wVim_c = wcols[:, 3:4]

            nhln2 = cst.tile([NN, 1], F32)
            v.memset(nhln2[:], -0.5 * LN2)
            c_one = cst.tile([NN, 1], F32)
            v.memset(c_one[:], 1.0)
            c_negk = cst.tile([NN, 1], F32)
            v.memset(c_negk[:], -(1.0 - EPS))
            c_eps = cst.tile([NN, 1], F32)
            v.memset(c_eps[:], EPS)
            c_onep = cst.tile([NN, 1], F32)
            v.memset(c_onep[:], 1.0 + EPS)

            def reduce_sum(wcol, rhs, nm):
                red = pspool.tile([1, BLOC], F32, tag="pr", name=nm)
                for c0 in range(0, BLOC, 512):
                    nc.tensor.matmul(red[0:1, c0:c0 + 512], wcol,
                                     rhs[:, c0:c0 + 512], start=True, stop=True)
                return red

            # ================ L pass ================
            pa_ps = pspool.tile([NN, BLOC], F32, tag="pa", name="paL")
            pb_ps = pspool.tile([NN, BLOC], F32, tag="pb", name="pbL")
            for c0 in range(0, BLOC, 512):
                nc.tensor.matmul(pa_ps[:, c0:c0 + 512], upL[:], rhs_pa[:, c0:c0 + 512],
                                 start=True, stop=True)
            for c0 in range(0, BLOC, 512):
                nc.tensor.matmul(pb_ps[:, c0:c0 + 512], upL[:], rhs_pb[:, c0:c0 + 512],
                                 start=True, stop=True)

            e_a2 = W("a0")
            sc.activation(e_a2[:], pa_ps[:], AF.Exp)
            omz4 = W("a2")
            v.tensor_scalar(omz4[:], zs4b[:], nu4L_c, 1.0, OP.mult, OP.add)
            X = W("a4")
            gp.tensor_tensor(X[:], omz4[:], e_a2[:], OP.mult)
            n2 = W("a6")
            v._custom_dve(_AFFSQ, out=n2[:], in0=c1b[:], s0=u4L_c, s1=EPS,
                          imm2=EPS2)
            rn2 = W("a5")
            v.reciprocal_approx_fast(rn2[:], n2[:])
            t_ = W("a6")
            v.tensor_tensor(t_[:], X[:], rn2[:], OP.mult)
            tDp = W("a4")
            v._custom_dve(_MULAFF, out=tDp[:], in0=t_[:], in1=c1b[:], s0=u4L_c,
                          s1=EPS)
            sgn = W("a8", dt=BF16)
            sc.activation(sgn[:], t_[:], AF.Sign, bias=c_one[:, 0:1], scale=-1.0)
            sqre = W("a5")
            sc.activation(sqre[:], tDp[:], AF.Square, bias=c_negk[:, 0:1], scale=1.0)
            sqim = W("a9")
            sc.activation(sqim[:], t_[:], AF.Square, bias=c_eps[:, 0:1], scale=-EPS)
            r2s = W("a6")
            gp.tensor_tensor(r2s[:], sqre[:], sqim[:], OP.add)
            lnom = W("a7")
            sc.activation(lnom[:], omz4[:], AF.Ln)
            lnr2s = W("a5")
            sc.activation(lnr2s[:], r2s[:], AF.Ln)
            r_ = W("aA", dt=BF16)
            sc.activation(r_[:], lnr2s[:], AF.Exp, scale=0.5)
            absre = W("a2", dt=BF16)
            sc.activation(absre[:], tDp[:], AF.Abs, bias=c_negk[:, 0:1], scale=1.0)
            rlg = W("a3", dt=BF16)
            v.tensor_tensor(rlg[:], absre[:], r_[:], OP.add)
            lnrlg = W("aB")
            sc.activation(lnrlg[:], rlg[:], AF.Ln)
            base = W("a0")
            v.tensor_tensor(base[:], pb_ps[:], lnom[:], OP.subtract)
            base2 = W("a2")
            v.tensor_tensor(base2[:], base[:], lnr2s[:], OP.subtract)
            lnim2 = W("a6")
            sc.activation(lnim2[:], sqim[:], AF.Ln)
            lnglg = W("a5")
            v.tensor_tensor(lnglg[:], base2[:], lnrlg[:], OP.add)
            SS = W("a1", dt=BF16)            # -> becomes igq after swap
            sc.activation(SS[:], lnglg[:], AF.Exp, bias=nhln2[:, 0:1], scale=0.5)
            prt = W("a3")
            gp.tensor_tensor(prt[:], base2[:], lnrlg[:], OP.subtract)
            lngsm = W("a0")
            v.tensor_tensor(lngsm[:], prt[:], lnim2[:], OP.add)
            TTs = W("a7", dt=BF16)           # -> becomes igre after swap
            sc.activation(TTs[:], lngsm[:], AF.Exp, bias=nhln2[:, 0:1], scale=0.5)
            TTs2 = W("a9", dt=BF16)
            v.tensor_copy(TTs2[:], TTs[:])
            m = W("aA", dt=BF16)
            gp.tensor_scalar(m[:], tDp[:], 1.0 - EPS, None, OP.is_ge)
            v.copy_predicated(TTs[:], m[:].bitcast(U16), SS[:])
            v.copy_predicated(SS[:], m[:].bitcast(U16), TTs2[:])
            igqs = W("a2", dt=BF16)
            v.tensor_tensor(igqs[:], SS[:], sgn[:], OP.mult)
            redLre = reduce_sum(wLre_c, TTs, "redLre")
            redLim = reduce_sum(wLim_c, igqs, "redLim")
            outLre = cst.tile([1, BLOC], F32)
            sc.activation(outLre[:], redLre[0:1, :], AF.Copy)
            nc.sync.dma_start(out=out_d[0, :].rearrange("(o n) -> o n", o=1),
                              in_=outLre[:])
            outLim = cst.tile([1, BLOC], F32)
            sc.activation(outLim[:], redLim[0:1, :], AF.Copy)
            nc.sync.dma_start(out=out_d[1, :].rearrange("(o n) -> o n", o=1),
                              in_=outLim[:])

            # ================ V pass ================
            pa2_ps = pspool.tile([NN, BLOC], F32, tag="pa", name="paV")
            pab_ps = pspool.tile([NN, BLOC], F32, tag="pb", name="pabV")
            for c0 in range(0, BLOC, 512):
                nc.tensor.matmul(pa2_ps[:, c0:c0 + 512], upV[:], rhs_pa[:, c0:c0 + 512],
                                 start=True, stop=True)
            for c0 in range(0, BLOC, 512):
                nc.tensor.matmul(pab_ps[:, c0:c0 + 512], upV[:],
                                 rhs_pab[:, c0:c0 + 512], start=True, stop=True)

            e_a2v = W("b0")
            sc.activation(e_a2v[:], pa2_ps[:], AF.Exp)
            omz4v = W("b2")
            v.tensor_scalar(omz4v[:], zs4b[:], nu4V_c, 1.0, OP.mult, OP.add)
            Y = W("b3")
            v.tensor_scalar(Y[:], c1b[:], u4V_c, None, OP.mult)
            Xv = W("b4")
            gp.tensor_tensor(Xv[:], omz4v[:], e_a2v[:], OP.mult)
            n2v = W("b0")
            v._custom_dve(_AFFSQ, out=n2v[:], in0=Xv[:], s0=1.0, s1=EPS,
                          imm2=EPS2)
            rn2v = W("b2")
            v.reciprocal_approx_fast(rn2v[:], n2v[:])
            t2 = W("b0")
            v.tensor_tensor(t2[:], Y[:], rn2v[:], OP.mult)
            t2Xp = W("b3")
            v._custom_dve(_MULAFF, out=t2Xp[:], in0=t2[:], in1=Xv[:], s0=1.0,
                          s1=EPS)
            sqre2 = W("b0")
            sc.activation(sqre2[:], t2Xp[:], AF.Square, bias=c_onep[:, 0:1], scale=-1.0)
            sqim2 = W("b2")
            sc.activation(sqim2[:], t2[:], AF.Square, bias=c_eps[:, 0:1], scale=EPS)
            r2s2 = W("b4")
            gp.tensor_tensor(r2s2[:], sqre2[:], sqim2[:], OP.add)
            lnr2s2 = W("b0")
            sc.activation(lnr2s2[:], r2s2[:], AF.Ln)
            r2v = W("b5", dt=BF16)
            sc.activation(r2v[:], lnr2s2[:], AF.Exp, scale=0.5)
            absre2 = W("b6", dt=BF16)
            sc.activation(absre2[:], t2Xp[:], AF.Abs, bias=c_onep[:, 0:1], scale=-1.0)
            rlg2 = W("b1", dt=BF16)
            v.tensor_tensor(rlg2[:], absre2[:], r2v[:], OP.add)
            lnrlg2 = W("b5")
            sc.activation(lnrlg2[:], rlg2[:], AF.Ln)
            base2v = W("b2")
            v.tensor_tensor(base2v[:], pab_ps[:], lnr2s2[:], OP.subtract)
            lnim2v = W("b6")
            sc.activation(lnim2v[:], sqim2[:], AF.Ln)
            lnglg2 = W("b4")
            v.tensor_tensor(lnglg2[:], base2v[:], lnrlg2[:], OP.add)
            SSv = W("b0")                    # -> becomes M2 after swap
            sc.activation(SSv[:], lnglg2[:], AF.Exp, bias=nhln2[:, 0:1], scale=0.5)
            prt_v = W("b1")
            gp.tensor_tensor(prt_v[:], base2v[:], lnrlg2[:], OP.subtract)
            lngsm2 = W("b4")
            v.tensor_tensor(lngsm2[:], prt_v[:], lnim2v[:], OP.add)
            TTv = W("b2")                    # -> becomes P2 after swap
            sc.activation(TTv[:], lngsm2[:], AF.Exp, bias=nhln2[:, 0:1], scale=0.5)
            TTv2 = W("b5")
            sc.activation(TTv2[:], TTv[:], AF.Copy)
            m2 = W("b1", dt=BF16)
            gp.tensor_scalar(m2[:], t2Xp[:], 1.0 + EPS, None, OP.is_le)
            v.copy_predicated(TTv[:], m2[:].bitcast(U16), SSv[:])
            v.copy_predicated(SSv[:], m2[:].bitcast(U16), TTv2[:])
            P2 = TTv
            M2 = SSv

            zdb = W("b3", dt=BF16)
            v.tensor_scalar(zdb[:], zs2b[:], u2V_c, EPS, OP.mult, OP.add)
            ndn = W("b5")
            v._custom_dve(_AFFSQ, out=ndn[:], in0=zdb[:], s0=1.0, s1=0.0,
                          imm2=EPS2)
            rndr = W("b4")
            v.reciprocal_approx_fast(rndr[:], ndn[:])

            P2m = W("b6", dt=BF16)
            v.tensor_scalar(P2m[:], P2[:], -1.0, None, OP.add)
            M2b = W("b5", dt=BF16)
            sc.activation(M2b[:], M2[:], AF.Copy)
            A12 = W("b1", dt=BF16)
            v.tensor_tensor(A12[:], P2m[:], zdb[:], OP.mult)
            A4 = W("b2", dt=BF16)
            v.scalar_tensor_tensor(A4[:], M2b[:], -EPS, A12[:], OP.mult, OP.add)
            igre = W("b1", dt=BF16)
            v.tensor_tensor(igre[:], A4[:], rndr[:], OP.mult)
            B1 = W("b0", dt=BF16)
            v.tensor_tensor(B1[:], M2b[:], zdb[:], OP.mult)
            B3 = W("b3", dt=BF16)
            v.scalar_tensor_tensor(B3[:], P2m[:], EPS, B1[:], OP.mult, OP.add)
            igim = W("b2", dt=BF16)
            v.tensor_tensor(igim[:], B3[:], rndr[:], OP.mult)
            redVre = reduce_sum(wVre_c, igre, "redVre")
            redVim = reduce_sum(wVim_c, igim, "redVim")

            # ---------------- finals ----------------
            Vr1 = R("f0", nm="Vr1")
            v.tensor_tensor(Vr1[:], redVre[0:1, :], zrow[:], OP.mult)
            outVre = cst.tile([1, BLOC], F32)
            v.scalar_tensor_tensor(outVre[:], invzrow[:], -2.0 * math.pi, Vr1[:],
                                   OP.mult, OP.add)
            nc.sync.dma_start(out=out_d[2, :].rearrange("(o n) -> o n", o=1),
                              in_=outVre[:])
            outVim = cst.tile([1, BLOC], F32)
            v.tensor_tensor(outVim[:], redVim[0:1, :], zrow[:], OP.mult)
            nc.sync.dma_start(out=out_d[3, :].rearrange("(o n) -> o n", o=1),
                              in_=outVim[:])
    return nc


_NC_CACHE = {}


def _restrict_act_tables(nc):
    """Monkeypatch table-set selection to the one set that serves every
    activation this kernel uses (exp/ln/square/sign/abs/copy/identity) so
    the steady state has zero ACT_TABLE_LOADs."""
    import types
    from concourse.hw_specs import get_activation_tables

    def _patched(self):
        tables = [(k, (v if k == "natural_log_exp_and_others" else set()))
                  for k, v in get_activation_tables(self.m.arch).items()]
        bacc._bass_rust.insert_act_table_loads(self, tables)

    nc.insert_act_table_loads = types.MethodType(_patched, nc)


def kernel(a, b, zs):
    a = np.asarray(a, dtype=np.float32)
    b = np.asarray(b, dtype=np.float32)
    zs = np.asarray(zs, dtype=np.float32)
    if "nc" not in _NC_CACHE:
        nc0 = build_nc()
        _restrict_act_tables(nc0)
        nc0.finalize()
        _NC_CACHE["nc"] = nc0
    nc = _NC_CACHE["nc"]
    in_maps = [
        {"a": a, "b": b, "zs": zs[i * BLOC: (i + 1) * BLOC].copy()}
        for i in range(NCORES)
    ]
    res = bass_utils.run_bass_kernel_spmd(nc, in_maps, core_ids=list(range(NCORES)))
    out = np.concatenate([res.results[i]["out"] for i in range(NCORES)], axis=1)
    return out.astype(np.float32)


if __name__ == "__main__":
    rng = np.random.default_rng(0)
    out = kernel(
        rng.standard_normal(5).astype(np.float32),
        rng.standard_normal(5).astype(np.float32),
        (0.02 + 0.975 * rng.random(8192)).astype(np.float32),
    )
    print(out.shape, out.dtype, out[:, :3])
